# revision 5
# baseline (speedup 1.0000x reference)
"""Trainium2 Bass kernel for nn_AttentionBlock (B=4, H=W=64, C=64, GROUPS=32).

Math (reference):
    hn = GroupNorm(x; gamma, beta, 32 groups, eps=1e-3)
    q = hn@wq+bq ; k = hn@wk+bk ; v = hn@wv+bv
    att = softmax(q k^T / 8) over the 4096 spatial positions
    out = x + (att @ v) @ wo + bo

Sharding: data-parallel, 2 cores per batch image, each core owns 2048 of the
4096 queries but holds the full key/value set for its batch. No collectives.

Per-core algorithm (single NeuronCore, all fused on-chip):
  - GroupNorm stats via ones-matmuls on PE (per-channel sum / sum-of-squares,
    group pair-combine via tiny 0/1 matmuls). The GN affine is folded into the
    1x1-conv weights: W~ = diag(scale_c) @ W, b~ = gnbias @ W + b, so the
    normalized tensor is never materialized.
  - x is transposed on PE (identity matmuls) into xT [C=64 part, S free] so all
    projections contract channels on the partition dim.
  - k-bias is dropped: adding bk shifts every score of a query by a constant
    (bk . q), which softmax cancels exactly.
  - Scores are computed as ST[t, s] (keys on partitions) so that exp(ST) can be
    fed straight back to PE as the moving operand of the att@v matmul - no
    transpose of the attention matrix is ever needed.  Softmax is max-free:
    |score| <= ~3 for unit-normal inputs, exp() cannot overflow fp32, and
    softmax(x) == softmax(x - max) exactly in infinite precision.
  - v gets an appended ones-column, so the att@v matmul also produces the
    softmax denominator l[s] for free (row 64 of the accumulator).
  - The output projection runs on the *unnormalized* accumulator: (O/l)@wo ==
    (O@wo)/l since row-scaling commutes with right matmul. wo_aug also carries
    an extra column that passes l through, so a single per-partition
    reciprocal+scale finishes softmax, then residual + bo are added.

Layout trick: x arrives per-core in "p-major" form x_pm[p, n] = x_local[p*32+n]
so the 1 MB image loads as one DMA with 8 KB/partition contiguous runs, and the
PE transposes of its [128, 64] slices enumerate spatial positions in a permuted
order. Attention is permutation-invariant over keys, and the query permutation
is undone on the host. x_q holds the core's query rows in kernel order for the
residual path.
"""

import numpy as np

import concourse.bass as bass
import concourse.tile as tile
from concourse import bacc, mybir
from concourse.bass_utils import run_bass_kernel_spmd
from concourse.masks import make_identity

F32 = mybir.dt.float32
BF16 = mybir.dt.bfloat16
AF = mybir.ActivationFunctionType
ALU = mybir.AluOpType

B, H, W, C = 4, 64, 64, 64
S = H * W            # 4096 spatial positions per image
SQ = S // 2          # 2048 queries per core
GROUPS = 32
EPS = 1e-3
N_CHUNK = S // 128   # 32 [128,64] slices per image
NQ = SQ // 128       # 16 query chunks
N_STRIPE = SQ // 512  # 4 query stripes of 512
SCALE = float(C) ** -0.5  # 0.125


def build_kernel():
    nc = bacc.Bacc("TRN2", target_bir_lowering=False, debug=False)

    x_pm = nc.dram_tensor("x_pm", [128, N_CHUNK, 64], F32, kind="ExternalInput")
    x_q = nc.dram_tensor("x_q", [SQ, C], F32, kind="ExternalInput")
    gamma = nc.dram_tensor("gamma", [C], F32, kind="ExternalInput")
    beta = nc.dram_tensor("beta", [C], F32, kind="ExternalInput")
    wq_d = nc.dram_tensor("wq", [C, C], F32, kind="ExternalInput")
    bq_d = nc.dram_tensor("bq", [C], F32, kind="ExternalInput")
    wk_d = nc.dram_tensor("wk", [C, C], F32, kind="ExternalInput")
    wv_d = nc.dram_tensor("wv", [C, C], F32, kind="ExternalInput")
    bv_d = nc.dram_tensor("bv", [C], F32, kind="ExternalInput")
    wo_d = nc.dram_tensor("wo", [C, C], F32, kind="ExternalInput")
    bo_d = nc.dram_tensor("bo", [C], F32, kind="ExternalInput")
    out_d = nc.dram_tensor("out", [SQ, C], F32, kind="ExternalOutput")

    with tile.TileContext(nc) as tc:
        _emit(nc, tc, x_pm.ap(), x_q.ap(), gamma.ap(), beta.ap(), wq_d.ap(),
              bq_d.ap(), wk_d.ap(), wv_d.ap(), bv_d.ap(), wo_d.ap(), bo_d.ap(),
              out_d.ap())
    nc.compile()
    return nc


def _emit(nc, tc, x_pm, x_q, gamma, beta, wq_d, bq_d, wk_d, wv_d, bv_d, wo_d,
          bo_d, out_d):
    from contextlib import ExitStack

    ctx = ExitStack()
    with ctx:
        const = ctx.enter_context(tc.tile_pool(name="const", bufs=1))
        big = ctx.enter_context(tc.tile_pool(name="big", bufs=1))
        tiny = ctx.enter_context(tc.tile_pool(name="tiny", bufs=1))

        # ---- constants ----
        ident = const.tile([128, 128], F32)
        make_identity(nc, ident)
        ones_col = const.tile([128, 1], F32)
        nc.gpsimd.memset(ones_col, 1.0)
        ones_col_bf = const.tile([128, 1], BF16)
        nc.gpsimd.memset(ones_col_bf, 1.0)
        zbias = const.tile([128, 1], F32)
        nc.gpsimd.memset(zbias, 0.0)
        eps32 = const.tile([32, 1], F32)
        nc.gpsimd.memset(eps32, EPS)

        # pair matrices: P64[c,g] = 1 iff c//2 == g ; P32x64 = its transpose
        p64 = const.tile([64, 32], F32)
        nc.gpsimd.memset(p64, 1.0)
        nc.gpsimd.affine_select(out=p64, in_=p64, compare_op=ALU.is_ge,
                                fill=0.0, base=0, pattern=[[-2, 32]],
                                channel_multiplier=1)
        nc.gpsimd.affine_select(out=p64, in_=p64, compare_op=ALU.is_ge,
                                fill=0.0, base=1, pattern=[[2, 32]],
                                channel_multiplier=-1)
        p32x64 = const.tile([32, 64], F32)
        nc.gpsimd.memset(p32x64, 1.0)
        nc.gpsimd.affine_select(out=p32x64, in_=p32x64, compare_op=ALU.is_ge,
                                fill=0.0, base=0, pattern=[[1, 64]],
                                channel_multiplier=-2)
        nc.gpsimd.affine_select(out=p32x64, in_=p32x64, compare_op=ALU.is_ge,
                                fill=0.0, base=1, pattern=[[-1, 64]],
                                channel_multiplier=2)

        # ---- weight / param DMAs ----
        wq_aug = const.tile([65, 64], F32)   # [Wq ; bq]
        nc.sync.dma_start(out=wq_aug[0:64, :], in_=wq_d)
        nc.sync.dma_start(out=wq_aug[64:65, :], in_=bq_d.rearrange("(o c) -> o c", o=1))
        wk_sb = const.tile([64, 64], F32)
        nc.sync.dma_start(out=wk_sb, in_=wk_d)
        wv_aug = const.tile([65, 65], F32)   # [Wv ; bv] plus e64 column
        nc.sync.dma_start(out=wv_aug[0:64, 0:64], in_=wv_d)
        nc.sync.dma_start(out=wv_aug[64:65, 0:64], in_=bv_d.rearrange("(o c) -> o c", o=1))
        nc.gpsimd.memset(wv_aug[0:64, 64:65], 0.0)
        nc.gpsimd.memset(wv_aug[64:65, 64:65], 1.0)
        wo_aug = const.tile([65, 65], F32)   # [wo ; 0] plus e64 column (passes l)
        nc.sync.dma_start(out=wo_aug[0:64, 0:64], in_=wo_d)
        nc.gpsimd.memset(wo_aug[64:65, 0:64], 0.0)
        nc.gpsimd.memset(wo_aug[0:64, 64:65], 0.0)
        nc.gpsimd.memset(wo_aug[64:65, 64:65], 1.0)
        gamma_col = const.tile([64, 1], F32)
        nc.sync.dma_start(out=gamma_col, in_=gamma.rearrange("(c o) -> c o", o=1))
        beta_col = const.tile([64, 1], F32)
        nc.sync.dma_start(out=beta_col, in_=beta.rearrange("(c o) -> c o", o=1))
        bo_bcast = const.tile([128, 64], F32)
        nc.sync.dma_start(out=bo_bcast, in_=bo_d.rearrange("(o c) -> o c", o=1).to_broadcast([128, 64]))

        # ---- x loads ----
        x_pm_sb = big.tile([128, N_CHUNK, 64], F32)
        nc.sync.dma_start(out=x_pm_sb, in_=x_pm)
        xq_sb = big.tile([128, NQ, 64], F32)
        nc.sync.dma_start(out=xq_sb, in_=x_q.rearrange("(m p) c -> p m c", p=128))

        # ---- PSUM pools (8 banks total: 1 + 3 + 2 + 2) ----
        misc_ps = ctx.enter_context(tc.tile_pool(name="misc_ps", bufs=1, space="PSUM"))
        st_ps = ctx.enter_context(tc.tile_pool(name="st_ps", bufs=3, space="PSUM"))
        op_ps = ctx.enter_context(tc.tile_pool(name="op_ps", bufs=2, space="PSUM"))
        ot_ps = ctx.enter_context(tc.tile_pool(name="ot_ps", bufs=2, space="PSUM"))

        # ---- squares (bf16 is plenty for the variance term) ----
        xsq_sb = big.tile([128, N_CHUNK, 64], BF16)
        for h in range(4):
            nc.vector.tensor_mul(xsq_sb[:, 8 * h:8 * (h + 1), :],
                                 x_pm_sb[:, 8 * h:8 * (h + 1), :],
                                 x_pm_sb[:, 8 * h:8 * (h + 1), :])

        # ---- per-channel stats as columns: sums_ps[c,0]=sum x, [c,1]=sum x^2 ----
        sums_ps = misc_ps.tile([64, 2], F32, tag="misc")
        for n in range(N_CHUNK):
            nc.tensor.matmul(sums_ps[:, 0:1], lhsT=x_pm_sb[:, n, :],
                             rhs=ones_col, start=(n == 0), stop=(n == N_CHUNK - 1))
        for n in range(N_CHUNK):
            nc.tensor.matmul(sums_ps[:, 1:2], lhsT=xsq_sb[:, n, :],
                             rhs=ones_col_bf, start=(n == 0), stop=(n == N_CHUNK - 1))

        # ---- transposes: xT[c, j] with j = 128*n + p  (bf16) ----
        xT = big.tile([64, S], BF16)
        for n in range(N_CHUNK):
            tr = ot_ps.tile([64, 128], F32, tag="ot")
            nc.tensor.transpose(tr, x_pm_sb[:, n, :], ident)
            if n % 2 == 0:
                nc.scalar.copy(out=xT[:, 128 * n:128 * (n + 1)], in_=tr)
            else:
                nc.vector.tensor_copy(out=xT[:, 128 * n:128 * (n + 1)], in_=tr)

        # ---- group-norm scale/bias per channel (tiny ops) ----
        s_sb = tiny.tile([64, 2], F32)
        nc.vector.tensor_copy(s_sb, sums_ps)
        gpair = misc_ps.tile([32, 2], F32, tag="misc")
        nc.tensor.matmul(gpair, lhsT=p64, rhs=s_sb)   # group sums
        gm = tiny.tile([32, 2], F32)                  # mean, E[x^2] per group
        nc.vector.tensor_scalar_mul(gm, gpair, 1.0 / 8192.0)
        var = tiny.tile([32, 1], F32)
        nc.vector.tensor_mul(var, gm[:, 0:1], gm[:, 0:1])
        nc.vector.tensor_sub(var, gm[:, 1:2], var)
        std = tiny.tile([32, 1], F32)
        nc.scalar.activation(std, var, AF.Sqrt, bias=eps32, scale=1.0)
        rstd = tiny.tile([32, 1], F32)
        nc.vector.reciprocal(rstd, std)
        packed32 = tiny.tile([32, 2], F32)            # [rstd | mean] per group
        nc.vector.tensor_copy(packed32[:, 0:1], rstd)
        nc.vector.tensor_copy(packed32[:, 1:2], gm[:, 0:1])
        chan = misc_ps.tile([64, 2], F32, tag="misc")  # expand groups->channels
        nc.tensor.matmul(chan, lhsT=p32x64, rhs=packed32)
        scale_col = tiny.tile([64, 1], F32)           # rstd_g * gamma_c
        nc.vector.tensor_mul(scale_col, chan[:, 0:1], gamma_col)
        gnbias = tiny.tile([65, 1], F32)              # beta - mean*scale, aug 1
        nc.vector.tensor_mul(gnbias[0:64, :], chan[:, 1:2], scale_col)
        nc.vector.tensor_sub(gnbias[0:64, :], beta_col, gnbias[0:64, :])
        nc.gpsimd.memset(gnbias[64:65, :], 1.0)

        # ---- fold GN into projection weights ----
        wq_sc = tiny.tile([64, 64], BF16)
        nc.vector.tensor_scalar_mul(wq_sc, wq_aug[0:64, :], scale_col)
        wk_sc = tiny.tile([64, 64], BF16)
        nc.vector.tensor_scalar_mul(wk_sc, wk_sb, scale_col)
        wv_sc = tiny.tile([64, 65], BF16)
        nc.vector.tensor_scalar_mul(wv_sc[:, 0:64], wv_aug[0:64, 0:64], scale_col)
        nc.gpsimd.memset(wv_sc[:, 64:65], 0.0)

        bqp = misc_ps.tile([64, 1], F32, tag="misc")  # total q bias (column)
        nc.tensor.matmul(bqp, lhsT=wq_aug, rhs=gnbias)
        bq_col = tiny.tile([64, 1], F32)
        nc.vector.tensor_copy(bq_col, bqp)
        bvp = misc_ps.tile([1, 65], F32, tag="misc")  # total v bias (row, aug 1)
        nc.tensor.matmul(bvp, lhsT=gnbias, rhs=wv_aug)
        bv_row = tiny.tile([1, 65], F32)
        nc.vector.tensor_copy(bv_row, bvp)
        # partition-broadcast needs a DRAM source: bounce the row through HBM
        vb_stage = nc.dram_tensor("vb_stage", [65], F32).ap()
        nc.sync.dma_start(out=vb_stage.rearrange("(o c) -> o c", o=1), in_=bv_row)
        vb_bcast = const.tile([128, 65], F32)
        nc.sync.dma_start(out=vb_bcast,
                          in_=vb_stage.rearrange("(o c) -> o c", o=1).to_broadcast([128, 65]))

        # ---- projections ----
        kT = big.tile([64, S], BF16)
        for j in range(8):
            kp = st_ps.tile([64, 512], F32, tag="st")
            nc.tensor.matmul(kp, lhsT=wk_sc, rhs=xT[:, 512 * j:512 * (j + 1)])
            nc.scalar.copy(out=kT[:, 512 * j:512 * (j + 1)], in_=kp)
        qT = big.tile([64, SQ], BF16)
        for j in range(N_STRIPE):
            qp = st_ps.tile([64, 512], F32, tag="st")
            nc.tensor.matmul(qp, lhsT=wq_sc, rhs=xT[:, 512 * j:512 * (j + 1)])
            nc.vector.tensor_scalar_add(qT[:, 512 * j:512 * (j + 1)], qp, bq_col)
        v_sb = []
        for n in range(N_CHUNK):
            vp = op_ps.tile([128, 65], F32, tag="op")
            nc.tensor.matmul(vp, lhsT=xT[:, 128 * n:128 * (n + 1)], rhs=wv_sc)
            vt = big.tile([128, 65], BF16, tag="v", bufs=N_CHUNK)
            nc.vector.tensor_add(vt, vp, vb_bcast)
            v_sb.append(vt)

        # ---- residual base: x + bo ----
        xb_sb = big.tile([128, NQ, 64], F32)
        for m in range(NQ):
            nc.vector.tensor_add(xb_sb[:, m, :], xq_sb[:, m, :], bo_bcast)

        # ---- main attention loop ----
        p_pool = ctx.enter_context(tc.tile_pool(name="p_pool", bufs=4))
        ep_pool = ctx.enter_context(tc.tile_pool(name="ep_pool", bufs=3))
        for j in range(N_STRIPE):
            ot = ot_ps.tile([65, 512], F32, tag="ot")
            for i in range(N_CHUNK):
                st = st_ps.tile([128, 512], F32, tag="st")
                nc.tensor.matmul(st, lhsT=kT[:, 128 * i:128 * (i + 1)],
                                 rhs=qT[:, 512 * j:512 * (j + 1)])
                pt = p_pool.tile([128, 512], BF16, tag="p")
                nc.scalar.activation(pt, st, AF.Exp, bias=zbias, scale=SCALE)
                nc.tensor.matmul(ot, lhsT=v_sb[i], rhs=pt,
                                 start=(i == 0), stop=(i == N_CHUNK - 1))
            ot_sb = ep_pool.tile([65, 512], F32, tag="ot_sb", bufs=2)
            nc.vector.tensor_copy(ot_sb, ot)
            for m in range(4):
                op = op_ps.tile([128, 65], F32, tag="op")
                nc.tensor.matmul(op, lhsT=ot_sb[:, 128 * m:128 * (m + 1)],
                                 rhs=wo_aug)
                rl = ep_pool.tile([128, 1], F32, tag="rl")
                nc.vector.reciprocal(rl, op[:, 64:65])
                res = ep_pool.tile([128, 64], F32, tag="res")
                nc.vector.scalar_tensor_tensor(out=res, in0=op[:, 0:64],
                                               scalar=rl,
                                               in1=xb_sb[:, 4 * j + m, :],
                                               op0=ALU.mult, op1=ALU.add)
                base = 512 * j + 128 * m
                nc.sync.dma_start(out=out_d[base:base + 128, :], in_=res)


_NC_CACHE = {}


def _get_nc():
    if "nc" not in _NC_CACHE:
        _NC_CACHE["nc"] = build_kernel()
    return _NC_CACHE["nc"]


def _core_index_maps(h):
    """Index maps for query-half h of a batch image.

    glob: local row r -> global spatial row (row blocks of 16 swapped for h=1)
    r_local: kernel s-index j -> local row
    """
    r = np.arange(S)
    glob = (r // 32) * 32 + ((r % 32) + 16 * h) % 32
    j = np.arange(SQ)
    r_local = (j % 128) * 32 + (j // 128)
    return glob, r_local


def build_in_maps(x, gamma, beta, wq, bq, wk, wv, bv, wo, bo):
    """Per-core NEFF input dicts plus (batch, rows) scatter info per core."""
    x = np.asarray(x, dtype=np.float32)
    shared = {
        "gamma": np.asarray(gamma, np.float32),
        "beta": np.asarray(beta, np.float32),
        "wq": np.asarray(wq, np.float32), "bq": np.asarray(bq, np.float32),
        "wk": np.asarray(wk, np.float32),
        "wv": np.asarray(wv, np.float32), "bv": np.asarray(bv, np.float32),
        "wo": np.asarray(wo, np.float32), "bo": np.asarray(bo, np.float32),
    }
    xf = x.reshape(B, S, C)
    in_maps = []
    scatter = []
    for core in range(8):
        b, h = core // 2, core % 2
        glob, r_local = _core_index_maps(h)
        x_local = xf[b][glob]
        in_maps.append({
            "x_pm": np.ascontiguousarray(x_local.reshape(128, N_CHUNK, 64)),
            "x_q": np.ascontiguousarray(x_local[r_local]),
            **shared,
        })
        scatter.append((b, glob[r_local]))
    return in_maps, scatter


def _run(in_maps, scatter, **spmd_kwargs):
    nc = _get_nc()
    res = run_bass_kernel_spmd(nc, in_maps, core_ids=list(range(8)),
                               **spmd_kwargs)
    out = np.empty((B, S, C), np.float32)
    for core in range(8):
        b, rows = scatter[core]
        out[b][rows] = res.results[core]["out"]
    return out.reshape(B, H, W, C), res


def kernel(x, gamma, beta, wq, bq, wk, bk, wv, bv, wo, bo):
    # bk is provably a no-op: it shifts each query's scores by the constant
    # bk.q which softmax cancels, so it is not shipped to the device.
    in_maps, scatter = build_in_maps(x, gamma, beta, wq, bq, wk, wv, bv, wo, bo)
    out, _ = _run(in_maps, scatter)
    return out


# revision 6
# speedup vs baseline: 1.0014x; 1.0014x over previous
"""Trainium2 Bass kernel for nn_AttentionBlock (B=4, H=W=64, C=64, GROUPS=32).

Math (reference):
    hn = GroupNorm(x; gamma, beta, 32 groups, eps=1e-3)
    q = hn@wq+bq ; k = hn@wk+bk ; v = hn@wv+bv
    att = softmax(q k^T / 8) over the 4096 spatial positions
    out = x + (att @ v) @ wo + bo

Sharding: data-parallel, 2 cores per batch image, each core owns 2048 of the
4096 queries but holds the full key/value set for its batch. No collectives.

Per-core algorithm (single NeuronCore, all fused on-chip):
  - GroupNorm stats via ones-matmuls on PE (per-channel sum / sum-of-squares,
    group pair-combine via tiny 0/1 matmuls). The GN affine is folded into the
    1x1-conv weights: W~ = diag(scale_c) @ W, b~ = gnbias @ W + b, so the
    normalized tensor is never materialized.
  - x is transposed on PE (identity matmuls) into xT [C=64 part, S free] so all
    projections contract channels on the partition dim.
  - k-bias is dropped: adding bk shifts every score of a query by a constant
    (bk . q), which softmax cancels exactly.
  - Scores are computed as ST[t, s] (keys on partitions) so that exp(ST) can be
    fed straight back to PE as the moving operand of the att@v matmul - no
    transpose of the attention matrix is ever needed.  Softmax is max-free:
    |score| <= ~3 for unit-normal inputs, exp() cannot overflow fp32, and
    softmax(x) == softmax(x - max) exactly in infinite precision.
  - v gets an appended ones-column, so the att@v matmul also produces the
    softmax denominator l[s] for free (row 64 of the accumulator).
  - The output projection runs on the *unnormalized* accumulator: (O/l)@wo ==
    (O@wo)/l since row-scaling commutes with right matmul. wo_aug also carries
    an extra column that passes l through, so a single per-partition
    reciprocal+scale finishes softmax, then residual + bo are added.

Layout trick: x arrives per-core in "p-major" form x_pm[p, n] = x_local[p*32+n]
so the 1 MB image loads as one DMA with 8 KB/partition contiguous runs, and the
PE transposes of its [128, 64] slices enumerate spatial positions in a permuted
order. Attention is permutation-invariant over keys, and the query permutation
is undone on the host. x_q holds the core's query rows in kernel order for the
residual path.
"""

import numpy as np

import concourse.bass as bass
import concourse.tile as tile
from concourse import bacc, mybir
from concourse.bass_utils import run_bass_kernel_spmd
from concourse.masks import make_identity

F32 = mybir.dt.float32
BF16 = mybir.dt.bfloat16
AF = mybir.ActivationFunctionType
ALU = mybir.AluOpType

B, H, W, C = 4, 64, 64, 64
S = H * W            # 4096 spatial positions per image
SQ = S // 2          # 2048 queries per core
GROUPS = 32
EPS = 1e-3
N_CHUNK = S // 128   # 32 [128,64] slices per image
NQ = SQ // 128       # 16 query chunks
N_STRIPE = SQ // 512  # 4 query stripes of 512
SCALE = float(C) ** -0.5  # 0.125


def build_kernel():
    nc = bacc.Bacc("TRN2", target_bir_lowering=False, debug=False)

    x_pm = nc.dram_tensor("x_pm", [128, N_CHUNK, 64], F32, kind="ExternalInput")
    x_q = nc.dram_tensor("x_q", [SQ, C], F32, kind="ExternalInput")
    gamma = nc.dram_tensor("gamma", [C], F32, kind="ExternalInput")
    beta = nc.dram_tensor("beta", [C], F32, kind="ExternalInput")
    wq_d = nc.dram_tensor("wq", [C, C], F32, kind="ExternalInput")
    bq_d = nc.dram_tensor("bq", [C], F32, kind="ExternalInput")
    wk_d = nc.dram_tensor("wk", [C, C], F32, kind="ExternalInput")
    wv_d = nc.dram_tensor("wv", [C, C], F32, kind="ExternalInput")
    bv_d = nc.dram_tensor("bv", [C], F32, kind="ExternalInput")
    wo_d = nc.dram_tensor("wo", [C, C], F32, kind="ExternalInput")
    bo_d = nc.dram_tensor("bo", [C], F32, kind="ExternalInput")
    out_d = nc.dram_tensor("out", [SQ, C], F32, kind="ExternalOutput")

    with tile.TileContext(nc) as tc:
        _emit(nc, tc, x_pm.ap(), x_q.ap(), gamma.ap(), beta.ap(), wq_d.ap(),
              bq_d.ap(), wk_d.ap(), wv_d.ap(), bv_d.ap(), wo_d.ap(), bo_d.ap(),
              out_d.ap())
    nc.compile()
    return nc


def _emit(nc, tc, x_pm, x_q, gamma, beta, wq_d, bq_d, wk_d, wv_d, bv_d, wo_d,
          bo_d, out_d):
    from contextlib import ExitStack

    ctx = ExitStack()
    with ctx:
        const = ctx.enter_context(tc.tile_pool(name="const", bufs=1))
        big = ctx.enter_context(tc.tile_pool(name="big", bufs=1))
        tiny = ctx.enter_context(tc.tile_pool(name="tiny", bufs=1))

        # ---- constants ----
        ident = const.tile([128, 128], F32)
        make_identity(nc, ident)
        ones_col = const.tile([128, 1], F32)
        nc.gpsimd.memset(ones_col, 1.0)
        ones_col_bf = const.tile([128, 1], BF16)
        nc.gpsimd.memset(ones_col_bf, 1.0)
        zbias = const.tile([128, 1], F32)
        nc.gpsimd.memset(zbias, 0.0)
        eps32 = const.tile([32, 1], F32)
        nc.gpsimd.memset(eps32, EPS)

        # pair matrices: P64[c,g] = 1 iff c//2 == g ; P32x64 = its transpose
        p64 = const.tile([64, 32], F32)
        nc.gpsimd.memset(p64, 1.0)
        nc.gpsimd.affine_select(out=p64, in_=p64, compare_op=ALU.is_ge,
                                fill=0.0, base=0, pattern=[[-2, 32]],
                                channel_multiplier=1)
        nc.gpsimd.affine_select(out=p64, in_=p64, compare_op=ALU.is_ge,
                                fill=0.0, base=1, pattern=[[2, 32]],
                                channel_multiplier=-1)
        p32x64 = const.tile([32, 64], F32)
        nc.gpsimd.memset(p32x64, 1.0)
        nc.gpsimd.affine_select(out=p32x64, in_=p32x64, compare_op=ALU.is_ge,
                                fill=0.0, base=0, pattern=[[1, 64]],
                                channel_multiplier=-2)
        nc.gpsimd.affine_select(out=p32x64, in_=p32x64, compare_op=ALU.is_ge,
                                fill=0.0, base=1, pattern=[[-1, 64]],
                                channel_multiplier=2)

        # ---- weight / param DMAs ----
        wq_aug = const.tile([65, 64], F32)   # [Wq ; bq]
        nc.sync.dma_start(out=wq_aug[0:64, :], in_=wq_d)
        nc.sync.dma_start(out=wq_aug[64:65, :], in_=bq_d.rearrange("(o c) -> o c", o=1))
        wk_sb = const.tile([64, 64], F32)
        nc.sync.dma_start(out=wk_sb, in_=wk_d)
        wv_aug = const.tile([65, 65], F32)   # [Wv ; bv] plus e64 column
        nc.sync.dma_start(out=wv_aug[0:64, 0:64], in_=wv_d)
        nc.sync.dma_start(out=wv_aug[64:65, 0:64], in_=bv_d.rearrange("(o c) -> o c", o=1))
        nc.gpsimd.memset(wv_aug[0:64, 64:65], 0.0)
        nc.gpsimd.memset(wv_aug[64:65, 64:65], 1.0)
        wo_aug = const.tile([65, 65], F32)   # [wo ; 0] plus e64 column (passes l)
        nc.sync.dma_start(out=wo_aug[0:64, 0:64], in_=wo_d)
        nc.gpsimd.memset(wo_aug[64:65, 0:64], 0.0)
        nc.gpsimd.memset(wo_aug[0:64, 64:65], 0.0)
        nc.gpsimd.memset(wo_aug[64:65, 64:65], 1.0)
        gamma_col = const.tile([64, 1], F32)
        nc.sync.dma_start(out=gamma_col, in_=gamma.rearrange("(c o) -> c o", o=1))
        beta_col = const.tile([64, 1], F32)
        nc.sync.dma_start(out=beta_col, in_=beta.rearrange("(c o) -> c o", o=1))
        bo_bcast = const.tile([128, 64], F32)
        nc.sync.dma_start(out=bo_bcast, in_=bo_d.rearrange("(o c) -> o c", o=1).to_broadcast([128, 64]))

        # ---- x loads ----
        x_pm_sb = big.tile([128, N_CHUNK, 64], F32)
        nc.sync.dma_start(out=x_pm_sb, in_=x_pm)
        xq_sb = big.tile([128, NQ, 64], F32)
        nc.sync.dma_start(out=xq_sb, in_=x_q.rearrange("(m p) c -> p m c", p=128))

        # ---- PSUM pools (8 banks total: 1 + 3 + 2 + 2) ----
        misc_ps = ctx.enter_context(tc.tile_pool(name="misc_ps", bufs=1, space="PSUM"))
        st_ps = ctx.enter_context(tc.tile_pool(name="st_ps", bufs=3, space="PSUM"))
        op_ps = ctx.enter_context(tc.tile_pool(name="op_ps", bufs=2, space="PSUM"))
        ot_ps = ctx.enter_context(tc.tile_pool(name="ot_ps", bufs=2, space="PSUM"))

        # ---- squares (bf16 is plenty for the variance term) ----
        xsq_sb = big.tile([128, N_CHUNK, 64], BF16)
        for h in range(4):
            nc.vector.tensor_mul(xsq_sb[:, 8 * h:8 * (h + 1), :],
                                 x_pm_sb[:, 8 * h:8 * (h + 1), :],
                                 x_pm_sb[:, 8 * h:8 * (h + 1), :])

        # ---- per-channel stats as columns: sums_ps[c,0]=sum x, [c,1]=sum x^2 ----
        sums_ps = misc_ps.tile([64, 2], F32, tag="misc")
        for n in range(N_CHUNK):
            nc.tensor.matmul(sums_ps[:, 0:1], lhsT=x_pm_sb[:, n, :],
                             rhs=ones_col, start=(n == 0), stop=(n == N_CHUNK - 1))
        for n in range(N_CHUNK):
            nc.tensor.matmul(sums_ps[:, 1:2], lhsT=xsq_sb[:, n, :],
                             rhs=ones_col_bf, start=(n == 0), stop=(n == N_CHUNK - 1))

        # ---- transposes: xT[c, j] with j = 128*n + p  (bf16) ----
        xT = big.tile([64, S], BF16)
        for n in range(N_CHUNK):
            tr = ot_ps.tile([64, 128], F32, tag="ot")
            nc.tensor.transpose(tr, x_pm_sb[:, n, :], ident)
            if n % 2 == 0:
                nc.scalar.copy(out=xT[:, 128 * n:128 * (n + 1)], in_=tr)
            else:
                nc.vector.tensor_copy(out=xT[:, 128 * n:128 * (n + 1)], in_=tr)

        # ---- group-norm scale/bias per channel (tiny ops) ----
        s_sb = tiny.tile([64, 2], F32)
        nc.vector.tensor_copy(s_sb, sums_ps)
        gpair = misc_ps.tile([32, 2], F32, tag="misc")
        nc.tensor.matmul(gpair, lhsT=p64, rhs=s_sb)   # group sums
        gm = tiny.tile([32, 2], F32)                  # mean, E[x^2] per group
        nc.vector.tensor_scalar_mul(gm, gpair, 1.0 / 8192.0)
        var = tiny.tile([32, 1], F32)
        nc.vector.tensor_mul(var, gm[:, 0:1], gm[:, 0:1])
        nc.vector.tensor_sub(var, gm[:, 1:2], var)
        std = tiny.tile([32, 1], F32)
        nc.scalar.activation(std, var, AF.Sqrt, bias=eps32, scale=1.0)
        rstd = tiny.tile([32, 1], F32)
        nc.vector.reciprocal(rstd, std)
        packed32 = tiny.tile([32, 2], F32)            # [rstd | mean] per group
        nc.vector.tensor_copy(packed32[:, 0:1], rstd)
        nc.vector.tensor_copy(packed32[:, 1:2], gm[:, 0:1])
        chan = misc_ps.tile([64, 2], F32, tag="misc")  # expand groups->channels
        nc.tensor.matmul(chan, lhsT=p32x64, rhs=packed32)
        scale_col = tiny.tile([64, 1], F32)           # rstd_g * gamma_c
        nc.vector.tensor_mul(scale_col, chan[:, 0:1], gamma_col)
        gnbias = tiny.tile([65, 1], F32)              # beta - mean*scale, aug 1
        nc.vector.tensor_mul(gnbias[0:64, :], chan[:, 1:2], scale_col)
        nc.vector.tensor_sub(gnbias[0:64, :], beta_col, gnbias[0:64, :])
        nc.gpsimd.memset(gnbias[64:65, :], 1.0)

        # ---- fold GN into projection weights ----
        wq_sc = tiny.tile([64, 64], BF16)
        nc.vector.tensor_scalar_mul(wq_sc, wq_aug[0:64, :], scale_col)
        wk_sc = tiny.tile([64, 64], BF16)
        nc.vector.tensor_scalar_mul(wk_sc, wk_sb, scale_col)
        wv_sc = tiny.tile([64, 65], BF16)
        nc.vector.tensor_scalar_mul(wv_sc[:, 0:64], wv_aug[0:64, 0:64], scale_col)
        nc.gpsimd.memset(wv_sc[:, 64:65], 0.0)

        bqp = misc_ps.tile([64, 1], F32, tag="misc")  # total q bias (column)
        nc.tensor.matmul(bqp, lhsT=wq_aug, rhs=gnbias)
        bq_col = tiny.tile([64, 1], F32)
        nc.vector.tensor_copy(bq_col, bqp)
        bvp = misc_ps.tile([1, 65], F32, tag="misc")  # total v bias (row, aug 1)
        nc.tensor.matmul(bvp, lhsT=gnbias, rhs=wv_aug)
        bv_row = tiny.tile([1, 65], F32)
        nc.vector.tensor_copy(bv_row, bvp)
        # partition-broadcast needs a DRAM source: bounce the row through HBM
        vb_stage = nc.dram_tensor("vb_stage", [65], F32).ap()
        nc.sync.dma_start(out=vb_stage.rearrange("(o c) -> o c", o=1), in_=bv_row)
        vb_bcast = const.tile([128, 65], F32)
        nc.sync.dma_start(out=vb_bcast,
                          in_=vb_stage.rearrange("(o c) -> o c", o=1).to_broadcast([128, 65]))

        # ---- projections ----
        kT = big.tile([64, S], BF16)
        for j in range(8):
            kp = st_ps.tile([64, 512], F32, tag="st")
            nc.tensor.matmul(kp, lhsT=wk_sc, rhs=xT[:, 512 * j:512 * (j + 1)])
            nc.scalar.copy(out=kT[:, 512 * j:512 * (j + 1)], in_=kp)
        qT = big.tile([64, SQ], BF16)
        for j in range(N_STRIPE):
            qp = st_ps.tile([64, 512], F32, tag="st")
            nc.tensor.matmul(qp, lhsT=wq_sc, rhs=xT[:, 512 * j:512 * (j + 1)])
            nc.vector.tensor_scalar_add(qT[:, 512 * j:512 * (j + 1)], qp, bq_col)
        v_sb = []
        for n in range(N_CHUNK):
            vp = op_ps.tile([128, 65], F32, tag="op")
            nc.tensor.matmul(vp, lhsT=xT[:, 128 * n:128 * (n + 1)], rhs=wv_sc)
            vt = big.tile([128, 65], BF16, tag="v", bufs=N_CHUNK)
            nc.vector.tensor_add(vt, vp, vb_bcast)
            v_sb.append(vt)

        # ---- residual base: x + bo ----
        xb_sb = big.tile([128, NQ, 64], F32)
        for m in range(NQ):
            nc.vector.tensor_add(xb_sb[:, m, :], xq_sb[:, m, :], bo_bcast)

        # ---- main attention loop ----
        # Software-pipelined by hand: the PE engine queue is strict FIFO, so
        # an att@v matmul that waits on its exp() would head-of-line-block the
        # next score matmul.  Skew: emit O-matmul i two iterations after its
        # score matmul, and emit each stripe's epilogue in the middle of the
        # NEXT stripe's loop so its PE work never waits at the queue head.
        p_pool = ctx.enter_context(tc.tile_pool(name="p_pool", bufs=6))
        ep_pool = ctx.enter_context(tc.tile_pool(name="ep_pool", bufs=3))
        SKEW = 2

        def make_epilogue(j, ot):
            def epi():
                ot_sb = ep_pool.tile([65, 512], F32, tag="ot_sb", bufs=2)
                nc.vector.tensor_copy(ot_sb, ot)
                for m in range(4):
                    op = op_ps.tile([128, 65], F32, tag="op")
                    nc.tensor.matmul(op, lhsT=ot_sb[:, 128 * m:128 * (m + 1)],
                                     rhs=wo_aug)
                    rl = ep_pool.tile([128, 1], F32, tag="rl")
                    nc.vector.reciprocal(rl, op[:, 64:65])
                    res = ep_pool.tile([128, 64], F32, tag="res")
                    nc.vector.scalar_tensor_tensor(out=res, in0=op[:, 0:64],
                                                   scalar=rl,
                                                   in1=xb_sb[:, 4 * j + m, :],
                                                   op0=ALU.mult, op1=ALU.add)
                    base = 512 * j + 128 * m
                    nc.sync.dma_start(out=out_d[base:base + 128, :], in_=res)
            return epi

        pending_epilogue = None
        for j in range(N_STRIPE):
            ot = ot_ps.tile([65, 512], F32, tag="ot")
            pts = {}
            for i in range(N_CHUNK + SKEW):
                if i < N_CHUNK:
                    st = st_ps.tile([128, 512], F32, tag="st")
                    nc.tensor.matmul(st, lhsT=kT[:, 128 * i:128 * (i + 1)],
                                     rhs=qT[:, 512 * j:512 * (j + 1)])
                    pt = p_pool.tile([128, 512], BF16, tag="p")
                    nc.scalar.activation(pt, st, AF.Exp, bias=zbias, scale=SCALE)
                    pts[i] = pt
                if i == 6 and pending_epilogue is not None:
                    pending_epilogue()
                    pending_epilogue = None
                io = i - SKEW
                if io >= 0:
                    nc.tensor.matmul(ot, lhsT=v_sb[io], rhs=pts.pop(io),
                                     start=(io == 0), stop=(io == N_CHUNK - 1))
            pending_epilogue = make_epilogue(j, ot)
        pending_epilogue()


_NC_CACHE = {}


def _get_nc():
    if "nc" not in _NC_CACHE:
        _NC_CACHE["nc"] = build_kernel()
    return _NC_CACHE["nc"]


def _core_index_maps(h):
    """Index maps for query-half h of a batch image.

    glob: local row r -> global spatial row (row blocks of 16 swapped for h=1)
    r_local: kernel s-index j -> local row
    """
    r = np.arange(S)
    glob = (r // 32) * 32 + ((r % 32) + 16 * h) % 32
    j = np.arange(SQ)
    r_local = (j % 128) * 32 + (j // 128)
    return glob, r_local


def build_in_maps(x, gamma, beta, wq, bq, wk, wv, bv, wo, bo):
    """Per-core NEFF input dicts plus (batch, rows) scatter info per core."""
    x = np.asarray(x, dtype=np.float32)
    shared = {
        "gamma": np.asarray(gamma, np.float32),
        "beta": np.asarray(beta, np.float32),
        "wq": np.asarray(wq, np.float32), "bq": np.asarray(bq, np.float32),
        "wk": np.asarray(wk, np.float32),
        "wv": np.asarray(wv, np.float32), "bv": np.asarray(bv, np.float32),
        "wo": np.asarray(wo, np.float32), "bo": np.asarray(bo, np.float32),
    }
    xf = x.reshape(B, S, C)
    in_maps = []
    scatter = []
    for core in range(8):
        b, h = core // 2, core % 2
        glob, r_local = _core_index_maps(h)
        x_local = xf[b][glob]
        in_maps.append({
            "x_pm": np.ascontiguousarray(x_local.reshape(128, N_CHUNK, 64)),
            "x_q": np.ascontiguousarray(x_local[r_local]),
            **shared,
        })
        scatter.append((b, glob[r_local]))
    return in_maps, scatter


def _run(in_maps, scatter, **spmd_kwargs):
    nc = _get_nc()
    res = run_bass_kernel_spmd(nc, in_maps, core_ids=list(range(8)),
                               **spmd_kwargs)
    out = np.empty((B, S, C), np.float32)
    for core in range(8):
        b, rows = scatter[core]
        out[b][rows] = res.results[core]["out"]
    return out.reshape(B, H, W, C), res


def kernel(x, gamma, beta, wq, bq, wk, bk, wv, bv, wo, bo):
    # bk is provably a no-op: it shifts each query's scores by the constant
    # bk.q which softmax cancels, so it is not shipped to the device.
    in_maps, scatter = build_in_maps(x, gamma, beta, wq, bq, wk, wv, bv, wo, bo)
    out, _ = _run(in_maps, scatter)
    return out


# revision 8
# speedup vs baseline: 1.6954x; 1.6930x over previous
"""Trainium2 Bass kernel for nn_AttentionBlock (B=4, H=W=64, C=64, GROUPS=32).

Math (reference):
    hn = GroupNorm(x; gamma, beta, 32 groups, eps=1e-3)
    q = hn@wq+bq ; k = hn@wk+bk ; v = hn@wv+bv
    att = softmax(q k^T / 8) over the 4096 spatial positions
    out = x + (att @ v) @ wo + bo

Sharding: data-parallel, 2 cores per batch image, each core owns 2048 of the
4096 queries but holds the full key/value set for its batch. No collectives.

Per-core pipeline (fully fused on one NeuronCore):
  - xT [C=64, S=4096] arrives pre-transposed in bf16 (host does the cheap
    numpy transpose+cast), so channel-contraction matmuls need no on-chip
    transposes. x_q keeps the core's own query rows in fp32 for the residual.
  - GroupNorm stats via bn_stats/bn_aggr per channel on DVE, then tiny 0/1
    matmuls pair-combine channels into groups and expand back. The GN affine
    folds into the projection weights: W~ = diag(scale_c)@W, b~ = gnbias@W + b.
  - k-bias is dropped: it shifts each query's scores by a constant, which
    softmax cancels exactly.
  - Scores are computed transposed, ST[t, s] (keys on partitions), so exp(ST)
    feeds the att@v matmul directly as the moving operand - the attention
    matrix is never transposed. Score matmuls have K=64, so two key-chunks run
    CONCURRENTLY on the two halves of the PE array (row-tiling), with kT/qT
    mirrored onto partitions 64:127 by one SBUF-to-SBUF DMA each.
  - Softmax is max-free: |score| <= ~3 for unit-normal inputs so exp cannot
    overflow, and softmax(x) == softmax(x - max) exactly.
  - exp() runs one ACT instruction per chunk-pair over a 2-bank PSUM tile to
    amortize the ~352-cycle activation pipeline latency.
  - v gets an appended ones-column so att@v also accumulates the softmax
    denominator l[s]. att@v is split into two K=64 halves accumulating into
    two PSUM banks (summed by one DVE add at stripe end): the halves run on
    opposite array halves, letting LDWEIGHTS overlap in-flight matmuls.
  - The output projection runs on the unnormalized accumulator ((O/l)@wo ==
    (O@wo)/l), with an extra wo column passing l through; one reciprocal +
    fused multiply-add applies softmax normalization, residual and bo.
"""

import numpy as np
import ml_dtypes

import concourse.bass as bass
import concourse.tile as tile
from concourse import bacc, mybir
from concourse.bass_utils import run_bass_kernel_spmd

F32 = mybir.dt.float32
BF16 = mybir.dt.bfloat16
AF = mybir.ActivationFunctionType
ALU = mybir.AluOpType

B, H, W, C = 4, 64, 64, 64
S = H * W            # 4096 spatial positions per image
SQ = S // 2          # 2048 queries per core
EPS = 1e-3
N_CHUNK = S // 128   # 32 key chunks
NQ = SQ // 128       # 16 query chunks
N_STRIPE = SQ // 512  # 4 query stripes
SCALE = float(C) ** -0.5  # 0.125


def build_kernel():
    nc = bacc.Bacc("TRN2", target_bir_lowering=False, debug=False)

    xT_d = nc.dram_tensor("xT", [C, S], BF16, kind="ExternalInput")
    x_q = nc.dram_tensor("x_q", [SQ, C], F32, kind="ExternalInput")
    gamma = nc.dram_tensor("gamma", [C], F32, kind="ExternalInput")
    beta = nc.dram_tensor("beta", [C], F32, kind="ExternalInput")
    wq_d = nc.dram_tensor("wq", [C, C], F32, kind="ExternalInput")
    bq_d = nc.dram_tensor("bq", [C], F32, kind="ExternalInput")
    wk_d = nc.dram_tensor("wk", [C, C], F32, kind="ExternalInput")
    wv_d = nc.dram_tensor("wv", [C, C], F32, kind="ExternalInput")
    bv_d = nc.dram_tensor("bv", [C], F32, kind="ExternalInput")
    wo_d = nc.dram_tensor("wo", [C, C], F32, kind="ExternalInput")
    bo_d = nc.dram_tensor("bo", [C], F32, kind="ExternalInput")
    out_d = nc.dram_tensor("out", [SQ, C], F32, kind="ExternalOutput")

    with tile.TileContext(nc) as tc:
        _emit(nc, tc, xT_d.ap(), x_q.ap(), gamma.ap(), beta.ap(), wq_d.ap(),
              bq_d.ap(), wk_d.ap(), wv_d.ap(), bv_d.ap(), wo_d.ap(), bo_d.ap(),
              out_d.ap())
    nc.compile()
    return nc


def _emit(nc, tc, xT_d, x_q, gamma, beta, wq_d, bq_d, wk_d, wv_d, bv_d, wo_d,
          bo_d, out_d):
    from contextlib import ExitStack

    ctx = ExitStack()
    with ctx:
        const = ctx.enter_context(tc.tile_pool(name="const", bufs=1))
        big = ctx.enter_context(tc.tile_pool(name="big", bufs=1))
        tiny = ctx.enter_context(tc.tile_pool(name="tiny", bufs=1))

        # ---- big input DMAs first (sync/HWDGE ring) ----
        xT = big.tile([64, S], BF16)
        nc.sync.dma_start(out=xT, in_=xT_d)
        xq_sb = big.tile([128, NQ, 64], F32)
        nc.sync.dma_start(out=xq_sb, in_=x_q.rearrange("(m p) c -> p m c", p=128))

        # ---- params via the scalar-engine HWDGE ring (parallel with sync) ----
        wq_aug = const.tile([65, 64], F32)   # [Wq ; bq]
        nc.scalar.dma_start(out=wq_aug[0:64, :], in_=wq_d)
        nc.scalar.dma_start(out=wq_aug[64:65, :], in_=bq_d.rearrange("(o c) -> o c", o=1))
        wk_sb = const.tile([64, 64], F32)
        nc.scalar.dma_start(out=wk_sb, in_=wk_d)
        wv_aug = const.tile([65, 65], F32)   # [Wv ; bv] plus e64 column
        nc.scalar.dma_start(out=wv_aug[0:64, 0:64], in_=wv_d)
        nc.scalar.dma_start(out=wv_aug[64:65, 0:64], in_=bv_d.rearrange("(o c) -> o c", o=1))
        nc.gpsimd.memset(wv_aug[0:64, 64:65], 0.0)
        nc.gpsimd.memset(wv_aug[64:65, 64:65], 1.0)
        wo_aug = const.tile([65, 65], BF16)  # [wo ; 0] plus e64 column (passes l)
        nc.gpsimd.dma_start(out=wo_aug[0:64, 0:64], in_=wo_d)  # SWDGE casts f32->bf16
        nc.gpsimd.memset(wo_aug[64:65, 0:64], 0.0)
        nc.gpsimd.memset(wo_aug[0:64, 64:65], 0.0)
        nc.gpsimd.memset(wo_aug[64:65, 64:65], 1.0)
        gamma_col = const.tile([64, 1], F32)
        nc.scalar.dma_start(out=gamma_col, in_=gamma.rearrange("(c o) -> c o", o=1))
        beta_col = const.tile([64, 1], F32)
        nc.scalar.dma_start(out=beta_col, in_=beta.rearrange("(c o) -> c o", o=1))
        bo_bcast = const.tile([128, 64], F32)
        nc.scalar.dma_start(out=bo_bcast, in_=bo_d.rearrange("(o c) -> o c", o=1).to_broadcast([128, 64]))

        zbias = const.tile([128, 1], F32)
        nc.gpsimd.memset(zbias, 0.0)
        eps32 = const.tile([32, 1], F32)
        nc.gpsimd.memset(eps32, EPS)

        # pair matrices: p64h[c,g] = 0.5 iff c//2 == g ; p32x64[g,c] = 1 iff c//2 == g
        p64h = const.tile([64, 32], F32)
        nc.gpsimd.memset(p64h, 0.5)
        nc.gpsimd.affine_select(out=p64h, in_=p64h, compare_op=ALU.is_ge,
                                fill=0.0, base=0, pattern=[[-2, 32]],
                                channel_multiplier=1)
        nc.gpsimd.affine_select(out=p64h, in_=p64h, compare_op=ALU.is_ge,
                                fill=0.0, base=1, pattern=[[2, 32]],
                                channel_multiplier=-1)
        p32x64 = const.tile([32, 64], F32)
        nc.gpsimd.memset(p32x64, 1.0)
        nc.gpsimd.affine_select(out=p32x64, in_=p32x64, compare_op=ALU.is_ge,
                                fill=0.0, base=0, pattern=[[1, 64]],
                                channel_multiplier=-2)
        nc.gpsimd.affine_select(out=p32x64, in_=p32x64, compare_op=ALU.is_ge,
                                fill=0.0, base=1, pattern=[[-1, 64]],
                                channel_multiplier=2)

        # ---- PSUM pools (8 banks: 2x2 + 2 + 2) ----
        st_ps = ctx.enter_context(tc.tile_pool(name="st_ps", bufs=2, space="PSUM"))
        ot_ps = ctx.enter_context(tc.tile_pool(name="ot_ps", bufs=2, space="PSUM"))
        op_ps = ctx.enter_context(tc.tile_pool(name="op_ps", bufs=2, space="PSUM"))

        # ---- GroupNorm stats on DVE: per-channel mean/var over all 4096 ----
        bstats = tiny.tile([64, 8, 6], F32)
        for i in range(8):
            nc.vector.bn_stats(bstats[:, i, :], xT[:, 512 * i:512 * (i + 1)])
        mv = tiny.tile([64, 2], F32)
        nc.vector.bn_aggr(mv, bstats)
        packed64 = tiny.tile([64, 2], F32)        # [mean_c, E[x^2]_c]
        nc.vector.tensor_copy(packed64[:, 0:1], mv[:, 0:1])
        nc.vector.tensor_mul(packed64[:, 1:2], mv[:, 0:1], mv[:, 0:1])
        nc.vector.tensor_add(packed64[:, 1:2], packed64[:, 1:2], mv[:, 1:2])
        gpair = op_ps.tile([32, 2], F32, tag="op")  # group [mean, E[x^2]]
        nc.tensor.matmul(gpair, lhsT=p64h, rhs=packed64)
        gm = tiny.tile([32, 2], F32)
        nc.vector.tensor_copy(gm, gpair)
        var = tiny.tile([32, 1], F32)
        nc.vector.tensor_mul(var, gm[:, 0:1], gm[:, 0:1])
        nc.vector.tensor_sub(var, gm[:, 1:2], var)
        std = tiny.tile([32, 1], F32)
        nc.scalar.activation(std, var, AF.Sqrt, bias=eps32, scale=1.0)
        rstd = tiny.tile([32, 1], F32)
        nc.vector.reciprocal(rstd, std)
        packed32 = tiny.tile([32, 2], F32)        # [rstd_g | mean_g]
        nc.vector.tensor_copy(packed32[:, 0:1], rstd)
        nc.vector.tensor_copy(packed32[:, 1:2], gm[:, 0:1])
        chan = op_ps.tile([64, 2], F32, tag="op")  # expand groups->channels
        nc.tensor.matmul(chan, lhsT=p32x64, rhs=packed32)
        scale_col = tiny.tile([64, 1], F32)       # rstd_g * gamma_c
        nc.vector.tensor_mul(scale_col, chan[:, 0:1], gamma_col)
        gnbias = tiny.tile([65, 1], F32)          # beta - mean*scale, aug 1
        nc.vector.tensor_mul(gnbias[0:64, :], chan[:, 1:2], scale_col)
        nc.vector.tensor_sub(gnbias[0:64, :], beta_col, gnbias[0:64, :])
        nc.gpsimd.memset(gnbias[64:65, :], 1.0)

        # ---- fold GN into projection weights ----
        wq_sc = tiny.tile([64, 64], BF16)
        nc.vector.tensor_scalar_mul(wq_sc, wq_aug[0:64, :], scale_col)
        wk_sc = tiny.tile([64, 64], BF16)
        nc.vector.tensor_scalar_mul(wk_sc, wk_sb, scale_col)
        wv_sc = tiny.tile([64, 65], BF16)
        nc.vector.tensor_scalar_mul(wv_sc[:, 0:64], wv_aug[0:64, 0:64], scale_col)
        nc.gpsimd.memset(wv_sc[:, 64:65], 0.0)

        bqp = op_ps.tile([64, 1], F32, tag="op")  # total q bias (column)
        nc.tensor.matmul(bqp, lhsT=wq_aug, rhs=gnbias)
        bq_col = tiny.tile([64, 1], F32)
        nc.vector.tensor_copy(bq_col, bqp)
        bvp = op_ps.tile([1, 65], F32, tag="op")  # total v bias (row, aug 1)
        nc.tensor.matmul(bvp, lhsT=gnbias, rhs=wv_aug)
        bv_row = tiny.tile([1, 65], F32)
        nc.vector.tensor_copy(bv_row, bvp)
        # partition-broadcast needs a DRAM source: bounce the row through HBM
        vb_stage = nc.dram_tensor("vb_stage", [65], F32).ap()
        nc.sync.dma_start(out=vb_stage.rearrange("(o c) -> o c", o=1), in_=bv_row)
        vb_bcast = const.tile([128, 65], F32)
        nc.sync.dma_start(out=vb_bcast,
                          in_=vb_stage.rearrange("(o c) -> o c", o=1).to_broadcast([128, 65]))

        # ---- projections; kT/qT mirrored to partitions 64:127 for row-tiling ----
        kT = big.tile([128, S], BF16)
        for j in range(8):
            kp = st_ps.tile([64, 512], F32, tag="st")
            nc.tensor.matmul(kp, lhsT=wk_sc, rhs=xT[:, 512 * j:512 * (j + 1)])
            nc.scalar.copy(out=kT[0:64, 512 * j:512 * (j + 1)], in_=kp)
        qT = big.tile([128, SQ], BF16)
        for j in range(N_STRIPE):
            qp = st_ps.tile([64, 512], F32, tag="st")
            nc.tensor.matmul(qp, lhsT=wq_sc, rhs=xT[:, 512 * j:512 * (j + 1)])
            nc.vector.tensor_scalar_add(qT[0:64, 512 * j:512 * (j + 1)], qp, bq_col)
        nc.sync.dma_start(out=kT[64:128, :], in_=kT[0:64, :])
        nc.sync.dma_start(out=qT[64:128, :], in_=qT[0:64, :])

        v_sb = []
        for n in range(N_CHUNK):
            vp = op_ps.tile([128, 65], F32, tag="op")
            nc.tensor.matmul(vp, lhsT=xT[:, 128 * n:128 * (n + 1)], rhs=wv_sc)
            vt = big.tile([128, 65], BF16, tag="v", bufs=N_CHUNK)
            nc.vector.tensor_add(vt, vp, vb_bcast)
            v_sb.append(vt)

        # ---- residual base: x + bo ----
        xb_sb = big.tile([128, NQ, 64], F32)
        for m in range(NQ):
            nc.vector.tensor_add(xb_sb[:, m, :], xq_sb[:, m, :], bo_bcast)

        # ---- main attention loop ----
        # Pairs of key chunks: the two K=64 score matmuls run concurrently on
        # the two row-halves of the PE array into the two banks of one PSUM
        # tile; exp covers both in one ACT instruction. att@v for each chunk
        # is split into two K=64 halves (lo/hi array rows) accumulating into
        # separate PSUM banks, summed once per stripe. All att@v work is
        # emitted one pair behind its exp so it never stalls the PE queue.
        p_pool = ctx.enter_context(tc.tile_pool(name="p_pool", bufs=3))
        ep_pool = ctx.enter_context(tc.tile_pool(name="ep_pool", bufs=3))
        N_PAIR = N_CHUNK // 2

        def emit_o(io, ot_lo, ot_hi, pt):
            nc.tensor.matmul(ot_lo, lhsT=v_sb[io][0:64, :], rhs=pt[0:64, :],
                             tile_position=(0, 0),
                             start=(io == 0), stop=(io == N_CHUNK - 1))
            nc.tensor.matmul(ot_hi, lhsT=v_sb[io][64:128, :], rhs=pt[64:128, :],
                             tile_position=(64, 0),
                             start=(io == 0), stop=(io == N_CHUNK - 1))

        def make_epilogue(j, ot_sb):
            def epi():
                for m in range(4):
                    op = op_ps.tile([128, 65], F32, tag="op")
                    nc.tensor.matmul(op, lhsT=ot_sb[:, 128 * m:128 * (m + 1)],
                                     rhs=wo_aug)
                    rl = ep_pool.tile([128, 1], F32, tag="rl")
                    nc.vector.reciprocal(rl, op[:, 64:65])
                    res = ep_pool.tile([128, 64], F32, tag="res")
                    nc.vector.scalar_tensor_tensor(out=res, in0=op[:, 0:64],
                                                   scalar=rl,
                                                   in1=xb_sb[:, 4 * j + m, :],
                                                   op0=ALU.mult, op1=ALU.add)
                    base = 512 * j + 128 * m
                    nc.sync.dma_start(out=out_d[base:base + 128, :], in_=res)
            return epi

        pending_epilogue = None
        for j in range(N_STRIPE):
            ot_lo = ot_ps.tile([65, 512], F32, tag="ot")
            ot_hi = ot_ps.tile([65, 512], F32, tag="ot")
            pts = {}
            for p in range(N_PAIR + 1):
                if p < N_PAIR:
                    i0, i1 = 2 * p, 2 * p + 1
                    st2 = st_ps.tile([128, 1024], F32, tag="st")
                    nc.tensor.matmul(st2[:, 0:512],
                                     lhsT=kT[0:64, 128 * i0:128 * (i0 + 1)],
                                     rhs=qT[0:64, 512 * j:512 * (j + 1)],
                                     tile_position=(0, 0))
                    nc.tensor.matmul(st2[:, 512:1024],
                                     lhsT=kT[64:128, 128 * i1:128 * (i1 + 1)],
                                     rhs=qT[64:128, 512 * j:512 * (j + 1)],
                                     tile_position=(64, 0))
                    pt = p_pool.tile([128, 1024], BF16, tag="p")
                    nc.scalar.activation(pt, st2, AF.Exp, bias=zbias, scale=SCALE)
                    pts[p] = pt
                if p == 3 and pending_epilogue is not None:
                    pending_epilogue()
                    pending_epilogue = None
                po = p - 1
                if po >= 0:
                    pt = pts.pop(po)
                    emit_o(2 * po, ot_lo, ot_hi, pt[:, 0:512])
                    emit_o(2 * po + 1, ot_lo, ot_hi, pt[:, 512:1024])
            # merge halves (+ l row); DVE may read only one PSUM input per op
            ot_sb = ep_pool.tile([65, 512], BF16, bufs=2, tag="ot_sb")
            nc.vector.tensor_copy(ot_sb, ot_lo)
            nc.vector.tensor_add(ot_sb, ot_sb, ot_hi)
            pending_epilogue = make_epilogue(j, ot_sb)
        pending_epilogue()


_NC_CACHE = {}


def _get_nc():
    if "nc" not in _NC_CACHE:
        _NC_CACHE["nc"] = build_kernel()
    return _NC_CACHE["nc"]


def build_in_maps(x, gamma, beta, wq, bq, wk, wv, bv, wo, bo):
    """Per-core NEFF input dicts plus (batch, rows) scatter info per core."""
    x = np.asarray(x, dtype=np.float32)
    shared = {
        "gamma": np.asarray(gamma, np.float32),
        "beta": np.asarray(beta, np.float32),
        "wq": np.asarray(wq, np.float32), "bq": np.asarray(bq, np.float32),
        "wk": np.asarray(wk, np.float32),
        "wv": np.asarray(wv, np.float32), "bv": np.asarray(bv, np.float32),
        "wo": np.asarray(wo, np.float32), "bo": np.asarray(bo, np.float32),
    }
    xf = x.reshape(B, S, C)
    in_maps = []
    scatter = []
    for core in range(8):
        b, h = core // 2, core % 2
        own = slice(h * SQ, (h + 1) * SQ)
        other = slice((1 - h) * SQ, (2 - h) * SQ)
        x_local = np.concatenate([xf[b][own], xf[b][other]], axis=0)
        in_maps.append({
            "xT": np.ascontiguousarray(x_local.T).astype(ml_dtypes.bfloat16),
            "x_q": np.ascontiguousarray(x_local[:SQ]),
            **shared,
        })
        scatter.append((b, np.arange(h * SQ, (h + 1) * SQ)))
    return in_maps, scatter


def _run(in_maps, scatter, **spmd_kwargs):
    nc = _get_nc()
    res = run_bass_kernel_spmd(nc, in_maps, core_ids=list(range(8)),
                               **spmd_kwargs)
    out = np.empty((B, S, C), np.float32)
    for core in range(8):
        b, rows = scatter[core]
        out[b][rows] = res.results[core]["out"]
    return out.reshape(B, H, W, C), res


def kernel(x, gamma, beta, wq, bq, wk, bk, wv, bv, wo, bo):
    # bk is provably a no-op: it shifts each query's scores by the constant
    # bk.q which softmax cancels, so it is not shipped to the device.
    in_maps, scatter = build_in_maps(x, gamma, beta, wq, bq, wk, wv, bv, wo, bo)
    out, _ = _run(in_maps, scatter)
    return out


# revision 16
# speedup vs baseline: 1.7433x; 1.0283x over previous
"""Trainium2 Bass kernel for nn_AttentionBlock (B=4, H=W=64, C=64, GROUPS=32).

Math (reference):
    hn = GroupNorm(x; gamma, beta, 32 groups, eps=1e-3)
    q = hn@wq+bq ; k = hn@wk+bk ; v = hn@wv+bv
    att = softmax(q k^T / 8) over the 4096 spatial positions
    out = x + (att @ v) @ wo + bo

Sharding: data-parallel, 2 cores per batch image, each core owns 2048 of the
4096 queries but holds the full key/value set for its batch. No collectives.

Per-core pipeline (fully fused on one NeuronCore):
  - xT [C=64, S=4096] arrives pre-transposed in bf16 (host does the cheap
    numpy transpose+cast), so channel-contraction matmuls need no on-chip
    transposes. x_q keeps the core's own query rows in fp32 for the residual.
  - GroupNorm stats via bn_stats/bn_aggr per channel on DVE, then tiny 0/1
    matmuls pair-combine channels into groups and expand back. The GN affine
    folds into the projection weights: W~ = diag(scale_c)@W, b~ = gnbias@W + b.
  - k-bias is dropped: it shifts each query's scores by a constant, which
    softmax cancels exactly.
  - Scores are computed transposed, ST[t, s] (keys on partitions), so exp(ST)
    feeds the att@v matmul directly as the moving operand - the attention
    matrix is never transposed. Score matmuls have K=64, so two key-chunks run
    CONCURRENTLY on the two halves of the PE array (row-tiling), with kT/qT
    mirrored onto partitions 64:127 by one SBUF-to-SBUF DMA each.
  - Softmax is max-free: |score| <= ~3 for unit-normal inputs so exp cannot
    overflow, and softmax(x) == softmax(x - max) exactly.
  - exp() runs one ACT instruction per chunk-pair over a 2-bank PSUM tile to
    amortize the ~352-cycle activation pipeline latency.
  - v gets an appended ones-column so att@v also accumulates the softmax
    denominator l[s]. att@v is split into two K=64 halves accumulating into
    two PSUM banks (summed by one DVE add at stripe end): the halves run on
    opposite array halves, letting LDWEIGHTS overlap in-flight matmuls.
  - The output projection runs on the unnormalized accumulator ((O/l)@wo ==
    (O@wo)/l), with an extra wo column passing l through; one reciprocal +
    fused multiply-add applies softmax normalization, residual and bo.
"""

import numpy as np
import ml_dtypes

import concourse.bass as bass
import concourse.tile as tile
from concourse import bacc, mybir
from concourse.bass_utils import run_bass_kernel_spmd

F32 = mybir.dt.float32
BF16 = mybir.dt.bfloat16
AF = mybir.ActivationFunctionType
ALU = mybir.AluOpType

B, H, W, C = 4, 64, 64, 64
S = H * W            # 4096 spatial positions per image
SQ = S // 2          # 2048 queries per core
EPS = 1e-3
N_CHUNK = S // 128   # 32 key chunks
NQ = SQ // 128       # 16 query chunks
N_STRIPE = SQ // 512  # 4 query stripes
SCALE = float(C) ** -0.5  # 0.125


def build_kernel():
    nc = bacc.Bacc("TRN2", target_bir_lowering=False, debug=False)

    xT_d = nc.dram_tensor("xT", [C, S], BF16, kind="ExternalInput")
    x_q = nc.dram_tensor("x_q", [SQ, C], F32, kind="ExternalInput")
    gamma = nc.dram_tensor("gamma", [C], F32, kind="ExternalInput")
    beta = nc.dram_tensor("beta", [C], F32, kind="ExternalInput")
    wq_d = nc.dram_tensor("wq", [C, C], F32, kind="ExternalInput")
    bq_d = nc.dram_tensor("bq", [C], F32, kind="ExternalInput")
    wk_d = nc.dram_tensor("wk", [C, C], F32, kind="ExternalInput")
    wv_d = nc.dram_tensor("wv", [C, C], F32, kind="ExternalInput")
    bv_d = nc.dram_tensor("bv", [C], F32, kind="ExternalInput")
    wo_d = nc.dram_tensor("wo", [C, C], F32, kind="ExternalInput")
    bo_d = nc.dram_tensor("bo", [C], F32, kind="ExternalInput")
    out_d = nc.dram_tensor("out", [SQ, C], F32, kind="ExternalOutput")

    with tile.TileContext(nc) as tc:
        _emit(nc, tc, xT_d.ap(), x_q.ap(), gamma.ap(), beta.ap(), wq_d.ap(),
              bq_d.ap(), wk_d.ap(), wv_d.ap(), bv_d.ap(), wo_d.ap(), bo_d.ap(),
              out_d.ap())
    nc.compile()
    return nc


def _emit(nc, tc, xT_d, x_q, gamma, beta, wq_d, bq_d, wk_d, wv_d, bv_d, wo_d,
          bo_d, out_d):
    from contextlib import ExitStack

    ctx = ExitStack()
    with ctx:
        const = ctx.enter_context(tc.tile_pool(name="const", bufs=1))
        big = ctx.enter_context(tc.tile_pool(name="big", bufs=1))
        tiny = ctx.enter_context(tc.tile_pool(name="tiny", bufs=1))

        # ---- big input DMAs first (sync/HWDGE ring), chunked so dependents
        # ---- can start before the full tensor lands ----
        xT = big.tile([65, S], BF16)  # row 64 = ones (feeds v's l-column)
        for i in range(4):
            nc.sync.dma_start(out=xT[0:64, 1024 * i:1024 * (i + 1)],
                              in_=xT_d[:, 1024 * i:1024 * (i + 1)])
        nc.gpsimd.memset(xT[64:65, :], 1.0)
        xq_sb = big.tile([128, NQ, 64], F32)
        nc.sync.dma_start(out=xq_sb, in_=x_q.rearrange("(m p) c -> p m c", p=128))

        # ---- params via the scalar-engine HWDGE ring (parallel with sync) ----
        wq_aug = const.tile([65, 64], F32)   # [Wq ; bq]
        nc.scalar.dma_start(out=wq_aug[0:64, :], in_=wq_d)
        nc.scalar.dma_start(out=wq_aug[64:65, :], in_=bq_d.rearrange("(o c) -> o c", o=1))
        wk_sb = const.tile([64, 64], F32)
        nc.scalar.dma_start(out=wk_sb, in_=wk_d)
        wv_aug = const.tile([65, 65], F32)   # [Wv ; bv] plus e64 column
        nc.scalar.dma_start(out=wv_aug[0:64, 0:64], in_=wv_d)
        nc.scalar.dma_start(out=wv_aug[64:65, 0:64], in_=bv_d.rearrange("(o c) -> o c", o=1))
        nc.gpsimd.memset(wv_aug[0:64, 64:65], 0.0)
        nc.gpsimd.memset(wv_aug[64:65, 64:65], 1.0)
        # wo_aug = [wo ; bvo] plus e64 column that passes l through. Row 64
        # multiplies the l-row of the accumulator, so after the division by l
        # it contributes the constant row bvo = bv_total @ wo - this is how the
        # v-bias is applied without ever materializing it per-position.
        wo_aug = const.tile([65, 65], BF16)
        nc.gpsimd.dma_start(out=wo_aug[0:64, 0:64], in_=wo_d)  # SWDGE casts f32->bf16
        nc.gpsimd.memset(wo_aug[0:64, 64:65], 0.0)
        nc.gpsimd.memset(wo_aug[64:65, 64:65], 1.0)
        wo_sb = const.tile([64, 64], F32)
        nc.scalar.dma_start(out=wo_sb, in_=wo_d)
        gamma_col = const.tile([64, 1], F32)
        nc.scalar.dma_start(out=gamma_col, in_=gamma.rearrange("(c o) -> c o", o=1))
        beta_col = const.tile([64, 1], F32)
        nc.scalar.dma_start(out=beta_col, in_=beta.rearrange("(c o) -> c o", o=1))
        bo_bcast = const.tile([128, 64], F32)
        nc.scalar.dma_start(out=bo_bcast, in_=bo_d.rearrange("(o c) -> o c", o=1).to_broadcast([128, 64]))

        zbias = const.tile([128, 1], F32)
        nc.gpsimd.memset(zbias, 0.0)
        eps32 = const.tile([32, 1], F32)
        nc.gpsimd.memset(eps32, EPS)
        # Preload the sqrt ACT table set while waiting on input DMAs so the
        # stats chain doesn't eat the ~1.3us table load.
        scratch1 = const.tile([1, 1], F32)
        nc.scalar.activation(scratch1, eps32[0:1, :], AF.Sqrt, bias=0.0, scale=1.0)

        # pair matrices: p64h[c,g] = 0.5 iff c//2 == g ; p32x64[g,c] = 1 iff c//2 == g
        p64h = const.tile([64, 32], F32)
        nc.gpsimd.memset(p64h, 0.5)
        nc.gpsimd.affine_select(out=p64h, in_=p64h, compare_op=ALU.is_ge,
                                fill=0.0, base=0, pattern=[[-2, 32]],
                                channel_multiplier=1)
        nc.gpsimd.affine_select(out=p64h, in_=p64h, compare_op=ALU.is_ge,
                                fill=0.0, base=1, pattern=[[2, 32]],
                                channel_multiplier=-1)
        p32x64 = const.tile([32, 64], F32)
        nc.gpsimd.memset(p32x64, 1.0)
        nc.gpsimd.affine_select(out=p32x64, in_=p32x64, compare_op=ALU.is_ge,
                                fill=0.0, base=0, pattern=[[1, 64]],
                                channel_multiplier=-2)
        nc.gpsimd.affine_select(out=p32x64, in_=p32x64, compare_op=ALU.is_ge,
                                fill=0.0, base=1, pattern=[[-1, 64]],
                                channel_multiplier=2)

        # ---- PSUM pools (8 banks: 2x2 + 2 + 2) ----
        st_ps = ctx.enter_context(tc.tile_pool(name="st_ps", bufs=2, space="PSUM"))
        ot_ps = ctx.enter_context(tc.tile_pool(name="ot_ps", bufs=2, space="PSUM"))
        op_ps = ctx.enter_context(tc.tile_pool(name="op_ps", bufs=2, space="PSUM"))

        # ---- GroupNorm stats on DVE: per-channel mean/var over all 4096 ----
        bstats = tiny.tile([64, 8, 6], F32)
        for i in range(8):
            nc.vector.bn_stats(bstats[:, i, :], xT[0:64, 512 * i:512 * (i + 1)])
        mv = tiny.tile([64, 2], F32)
        nc.vector.bn_aggr(mv, bstats)
        packed64 = tiny.tile([64, 2], F32)        # [mean_c, E[x^2]_c]
        nc.vector.tensor_copy(packed64[:, 0:1], mv[:, 0:1])
        nc.vector.tensor_mul(packed64[:, 1:2], mv[:, 0:1], mv[:, 0:1])
        nc.vector.tensor_add(packed64[:, 1:2], packed64[:, 1:2], mv[:, 1:2])
        gpair = op_ps.tile([32, 2], F32, tag="op")  # group [mean, E[x^2]]
        nc.tensor.matmul(gpair, lhsT=p64h, rhs=packed64)
        gm = tiny.tile([32, 2], F32)
        nc.vector.tensor_copy(gm, gpair)
        var = tiny.tile([32, 1], F32)
        nc.vector.tensor_mul(var, gm[:, 0:1], gm[:, 0:1])
        nc.vector.tensor_sub(var, gm[:, 1:2], var)
        std = tiny.tile([32, 1], F32)
        nc.scalar.activation(std, var, AF.Sqrt, bias=eps32, scale=1.0)
        # Swap in the exp table set now, before the main loop needs it.
        scratch2 = const.tile([1, 1], F32)
        nc.scalar.activation(scratch2, eps32[0:1, :], AF.Exp, bias=0.0, scale=1.0)
        rstd = tiny.tile([32, 1], F32)
        nc.vector.reciprocal(rstd, std)
        packed32 = tiny.tile([32, 2], F32)        # [rstd_g | mean_g]
        nc.vector.tensor_copy(packed32[:, 0:1], rstd)
        nc.vector.tensor_copy(packed32[:, 1:2], gm[:, 0:1])
        chan = op_ps.tile([64, 2], F32, tag="op")  # expand groups->channels
        nc.tensor.matmul(chan, lhsT=p32x64, rhs=packed32)
        scale_col = tiny.tile([64, 1], F32)       # rstd_g * gamma_c
        nc.vector.tensor_mul(scale_col, chan[:, 0:1], gamma_col)
        gnbias = tiny.tile([65, 1], F32)          # beta - mean*scale, aug 1
        nc.vector.tensor_mul(gnbias[0:64, :], chan[:, 1:2], scale_col)
        nc.vector.tensor_sub(gnbias[0:64, :], beta_col, gnbias[0:64, :])
        nc.gpsimd.memset(gnbias[64:65, :], 1.0)

        # ---- fold GN into projection weights ----
        wq_sc = tiny.tile([64, 64], BF16)
        nc.vector.tensor_scalar_mul(wq_sc, wq_aug[0:64, :], scale_col)
        wk_sc = tiny.tile([64, 64], BF16)
        nc.vector.tensor_scalar_mul(wk_sc, wk_sb, scale_col)
        # wv_sc rows 0:64 = scaled Wv (no bias); row 64 = e64 so that the ones
        # row of xT produces v's l-column.
        wv_sc = tiny.tile([65, 65], BF16)
        nc.vector.tensor_scalar_mul(wv_sc[0:64, 0:64], wv_aug[0:64, 0:64], scale_col)
        nc.gpsimd.memset(wv_sc[0:64, 64:65], 0.0)
        nc.gpsimd.memset(wv_sc[64:65, 0:64], 0.0)
        nc.gpsimd.memset(wv_sc[64:65, 64:65], 1.0)

        bqp = op_ps.tile([64, 1], F32, tag="op")  # total q bias (column)
        nc.tensor.matmul(bqp, lhsT=wq_aug, rhs=gnbias)
        bq_col = tiny.tile([64, 1], F32)
        nc.vector.tensor_copy(bq_col, bqp)
        # bvo row for wo_aug: bvo = (gnbias@Wv + bv) @ wo, bounced through HBM
        # to land on partition 64 (engines are lane-locked; DMA is not). This
        # only gates the first output projection, well off the critical path.
        bvcp = op_ps.tile([65, 1], F32, tag="op")
        nc.tensor.matmul(bvcp, lhsT=wv_aug, rhs=gnbias)
        bv_col = tiny.tile([64, 1], F32)
        nc.vector.tensor_copy(bv_col, bvcp[0:64, :])
        bvop = op_ps.tile([1, 64], F32, tag="op")
        nc.tensor.matmul(bvop, lhsT=bv_col, rhs=wo_sb)
        bvo_row = tiny.tile([1, 64], F32)
        nc.vector.tensor_copy(bvo_row, bvop)
        bvo_stage = nc.dram_tensor("bvo_stage", [64], F32).ap()
        nc.sync.dma_start(out=bvo_stage.rearrange("(o c) -> o c", o=1), in_=bvo_row)
        nc.gpsimd.dma_start(out=wo_aug[64:65, 0:64],
                            in_=bvo_stage.rearrange("(o c) -> o c", o=1))

        # ---- projections; kT/qT mirrored to partitions 64:127 for row-tiling ----
        kT = big.tile([128, S], BF16)
        qT = big.tile([128, SQ], BF16)
        for j in range(8):
            kp = st_ps.tile([64, 512], F32, tag="st")
            nc.tensor.matmul(kp, lhsT=wk_sc, rhs=xT[0:64, 512 * j:512 * (j + 1)])
            nc.scalar.copy(out=kT[0:64, 512 * j:512 * (j + 1)], in_=kp)
            nc.sync.dma_start(out=kT[64:128, 512 * j:512 * (j + 1)],
                              in_=kT[0:64, 512 * j:512 * (j + 1)])
            if j < N_STRIPE:
                qp = st_ps.tile([64, 512], F32, tag="st")
                nc.tensor.matmul(qp, lhsT=wq_sc, rhs=xT[0:64, 512 * j:512 * (j + 1)])
                nc.vector.tensor_scalar_add(qT[0:64, 512 * j:512 * (j + 1)], qp, bq_col)
                nc.sync.dma_start(out=qT[64:128, 512 * j:512 * (j + 1)],
                                  in_=qT[0:64, 512 * j:512 * (j + 1)])

        v_sb = []
        for n in range(N_CHUNK):
            vp = op_ps.tile([128, 65], F32, tag="op")
            nc.tensor.matmul(vp, lhsT=xT[:, 128 * n:128 * (n + 1)], rhs=wv_sc)
            vt = big.tile([128, 65], BF16, tag="v", bufs=N_CHUNK)
            nc.vector.tensor_copy(vt, vp)
            v_sb.append(vt)

        # ---- residual base: x + bo ----
        xb_sb = big.tile([128, NQ, 64], F32)
        for m in range(NQ):
            nc.vector.tensor_add(xb_sb[:, m, :], xq_sb[:, m, :], bo_bcast)

        # ---- main attention loop ----
        # Pairs of key chunks: the two K=64 score matmuls run concurrently on
        # the two row-halves of the PE array into the two banks of one PSUM
        # tile; exp covers both in one ACT instruction. att@v for each chunk
        # is split into two K=64 halves (lo/hi array rows) accumulating into
        # separate PSUM banks, summed once per stripe. All att@v work is
        # emitted one pair behind its exp so it never stalls the PE queue.
        p_pool = ctx.enter_context(tc.tile_pool(name="p_pool", bufs=3))
        ep_pool = ctx.enter_context(tc.tile_pool(name="ep_pool", bufs=3))
        N_PAIR = N_CHUNK // 2

        def emit_o(io, ot_lo, ot_hi, pt):
            nc.tensor.matmul(ot_lo, lhsT=v_sb[io][0:64, :], rhs=pt[0:64, :],
                             tile_position=(0, 0),
                             start=(io == 0), stop=(io == N_CHUNK - 1))
            nc.tensor.matmul(ot_hi, lhsT=v_sb[io][64:128, :], rhs=pt[64:128, :],
                             tile_position=(64, 0),
                             start=(io == 0), stop=(io == N_CHUNK - 1))

        def make_epilogue(j, ot_sb):
            def epi():
                for m in range(4):
                    op = op_ps.tile([128, 65], F32, tag="op")
                    nc.tensor.matmul(op, lhsT=ot_sb[:, 128 * m:128 * (m + 1)],
                                     rhs=wo_aug)
                    rl = ep_pool.tile([128, 1], F32, tag="rl")
                    nc.vector.reciprocal(rl, op[:, 64:65])
                    res = ep_pool.tile([128, 64], F32, tag="res")
                    nc.vector.scalar_tensor_tensor(out=res, in0=op[:, 0:64],
                                                   scalar=rl,
                                                   in1=xb_sb[:, 4 * j + m, :],
                                                   op0=ALU.mult, op1=ALU.add)
                    base = 512 * j + 128 * m
                    nc.sync.dma_start(out=out_d[base:base + 128, :], in_=res)
            return epi

        pending_epilogue = None
        for j in range(N_STRIPE):
            ot_lo = ot_ps.tile([65, 512], F32, tag="ot")
            ot_hi = ot_ps.tile([65, 512], F32, tag="ot")
            pts = {}
            for p in range(N_PAIR + 1):
                if p < N_PAIR:
                    i0, i1 = 2 * p, 2 * p + 1
                    st2 = st_ps.tile([128, 1024], F32, tag="st")
                    nc.tensor.matmul(st2[:, 0:512],
                                     lhsT=kT[0:64, 128 * i0:128 * (i0 + 1)],
                                     rhs=qT[0:64, 512 * j:512 * (j + 1)],
                                     tile_position=(0, 0))
                    nc.tensor.matmul(st2[:, 512:1024],
                                     lhsT=kT[64:128, 128 * i1:128 * (i1 + 1)],
                                     rhs=qT[64:128, 512 * j:512 * (j + 1)],
                                     tile_position=(64, 0))
                    pt = p_pool.tile([128, 1024], BF16, tag="p")
                    nc.scalar.activation(pt, st2, AF.Exp, bias=zbias, scale=SCALE)
                    pts[p] = pt
                if p == 3 and pending_epilogue is not None:
                    pending_epilogue()
                    pending_epilogue = None
                po = p - 1
                if po >= 0:
                    pt = pts.pop(po)
                    emit_o(2 * po, ot_lo, ot_hi, pt[:, 0:512])
                    emit_o(2 * po + 1, ot_lo, ot_hi, pt[:, 512:1024])
            # merge halves (+ l row); DVE may read only one PSUM input per op
            ot_sb = ep_pool.tile([65, 512], BF16, bufs=2, tag="ot_sb")
            nc.vector.tensor_copy(ot_sb, ot_lo)
            nc.vector.tensor_add(ot_sb, ot_sb, ot_hi)
            pending_epilogue = make_epilogue(j, ot_sb)
        pending_epilogue()


_NC_CACHE = {}


def _get_nc():
    if "nc" not in _NC_CACHE:
        _NC_CACHE["nc"] = build_kernel()
    return _NC_CACHE["nc"]


def build_in_maps(x, gamma, beta, wq, bq, wk, wv, bv, wo, bo):
    """Per-core NEFF input dicts plus (batch, rows) scatter info per core."""
    x = np.asarray(x, dtype=np.float32)
    shared = {
        "gamma": np.asarray(gamma, np.float32),
        "beta": np.asarray(beta, np.float32),
        "wq": np.asarray(wq, np.float32), "bq": np.asarray(bq, np.float32),
        "wk": np.asarray(wk, np.float32),
        "wv": np.asarray(wv, np.float32), "bv": np.asarray(bv, np.float32),
        "wo": np.asarray(wo, np.float32), "bo": np.asarray(bo, np.float32),
    }
    xf = x.reshape(B, S, C)
    in_maps = []
    scatter = []
    for core in range(8):
        b, h = core // 2, core % 2
        own = slice(h * SQ, (h + 1) * SQ)
        other = slice((1 - h) * SQ, (2 - h) * SQ)
        x_local = np.concatenate([xf[b][own], xf[b][other]], axis=0)
        in_maps.append({
            "xT": np.ascontiguousarray(x_local.T).astype(ml_dtypes.bfloat16),
            "x_q": np.ascontiguousarray(x_local[:SQ]),
            **shared,
        })
        scatter.append((b, np.arange(h * SQ, (h + 1) * SQ)))
    return in_maps, scatter


def _run(in_maps, scatter, **spmd_kwargs):
    nc = _get_nc()
    res = run_bass_kernel_spmd(nc, in_maps, core_ids=list(range(8)),
                               **spmd_kwargs)
    out = np.empty((B, S, C), np.float32)
    for core in range(8):
        b, rows = scatter[core]
        out[b][rows] = res.results[core]["out"]
    return out.reshape(B, H, W, C), res


def kernel(x, gamma, beta, wq, bq, wk, bk, wv, bv, wo, bo):
    # bk is provably a no-op: it shifts each query's scores by the constant
    # bk.q which softmax cancels, so it is not shipped to the device.
    in_maps, scatter = build_in_maps(x, gamma, beta, wq, bq, wk, wv, bv, wo, bo)
    out, _ = _run(in_maps, scatter)
    return out


# revision 17
# speedup vs baseline: 1.7905x; 1.0271x over previous
"""Trainium2 Bass kernel for nn_AttentionBlock (B=4, H=W=64, C=64, GROUPS=32).

Math (reference):
    hn = GroupNorm(x; gamma, beta, 32 groups, eps=1e-3)
    q = hn@wq+bq ; k = hn@wk+bk ; v = hn@wv+bv
    att = softmax(q k^T / 8) over the 4096 spatial positions
    out = x + (att @ v) @ wo + bo

Sharding: data-parallel, 2 cores per batch image, each core owns 2048 of the
4096 queries but holds the full key/value set for its batch. No collectives.

Per-core pipeline (fully fused on one NeuronCore):
  - xT [C=64, S=4096] arrives pre-transposed in bf16 (host does the cheap
    numpy transpose+cast), so channel-contraction matmuls need no on-chip
    transposes. x_q keeps the core's own query rows in fp32 for the residual.
  - GroupNorm stats via bn_stats/bn_aggr per channel on DVE, then tiny 0/1
    matmuls pair-combine channels into groups and expand back. The GN affine
    folds into the projection weights: W~ = diag(scale_c)@W, b~ = gnbias@W + b.
  - k-bias is dropped: it shifts each query's scores by a constant, which
    softmax cancels exactly.
  - Scores are computed transposed, ST[t, s] (keys on partitions), so exp(ST)
    feeds the att@v matmul directly as the moving operand - the attention
    matrix is never transposed. Score matmuls have K=64, so two key-chunks run
    CONCURRENTLY on the two halves of the PE array (row-tiling), with kT/qT
    mirrored onto partitions 64:127 by one SBUF-to-SBUF DMA each.
  - Softmax is max-free: |score| <= ~3 for unit-normal inputs so exp cannot
    overflow, and softmax(x) == softmax(x - max) exactly.
  - exp() runs one ACT instruction per chunk-pair over a 2-bank PSUM tile to
    amortize the ~352-cycle activation pipeline latency.
  - v gets an appended ones-column so att@v also accumulates the softmax
    denominator l[s]. att@v is split into two K=64 halves accumulating into
    two PSUM banks (summed by one DVE add at stripe end): the halves run on
    opposite array halves, letting LDWEIGHTS overlap in-flight matmuls.
  - The output projection runs on the unnormalized accumulator ((O/l)@wo ==
    (O@wo)/l), with an extra wo column passing l through; one reciprocal +
    fused multiply-add applies softmax normalization, residual and bo.
"""

import numpy as np
import ml_dtypes

import concourse.bass as bass
import concourse.tile as tile
from concourse import bacc, mybir
from concourse.bass_utils import run_bass_kernel_spmd

F32 = mybir.dt.float32
BF16 = mybir.dt.bfloat16
AF = mybir.ActivationFunctionType
ALU = mybir.AluOpType

B, H, W, C = 4, 64, 64, 64
S = H * W            # 4096 spatial positions per image
SQ = S // 2          # 2048 queries per core
EPS = 1e-3
N_CHUNK = S // 128   # 32 key chunks
NQ = SQ // 128       # 16 query chunks
N_STRIPE = SQ // 512  # 4 query stripes
SCALE = float(C) ** -0.5  # 0.125


def build_kernel():
    nc = bacc.Bacc("TRN2", target_bir_lowering=False, debug=False)

    xT_d = nc.dram_tensor("xT", [C, S], BF16, kind="ExternalInput")
    x_q = nc.dram_tensor("x_q", [SQ, C], F32, kind="ExternalInput")
    gamma = nc.dram_tensor("gamma", [C], F32, kind="ExternalInput")
    beta = nc.dram_tensor("beta", [C], F32, kind="ExternalInput")
    wq_d = nc.dram_tensor("wq", [C, C], F32, kind="ExternalInput")
    bq_d = nc.dram_tensor("bq", [C], F32, kind="ExternalInput")
    wk_d = nc.dram_tensor("wk", [C, C], F32, kind="ExternalInput")
    wv_d = nc.dram_tensor("wv", [C, C], F32, kind="ExternalInput")
    bv_d = nc.dram_tensor("bv", [C], F32, kind="ExternalInput")
    wo_d = nc.dram_tensor("wo", [C, C], F32, kind="ExternalInput")
    bo_d = nc.dram_tensor("bo", [C], F32, kind="ExternalInput")
    out_d = nc.dram_tensor("out", [SQ, C], F32, kind="ExternalOutput")

    with tile.TileContext(nc) as tc:
        _emit(nc, tc, xT_d.ap(), x_q.ap(), gamma.ap(), beta.ap(), wq_d.ap(),
              bq_d.ap(), wk_d.ap(), wv_d.ap(), bv_d.ap(), wo_d.ap(), bo_d.ap(),
              out_d.ap())
    nc.compile()
    return nc


def _emit(nc, tc, xT_d, x_q, gamma, beta, wq_d, bq_d, wk_d, wv_d, bv_d, wo_d,
          bo_d, out_d):
    from contextlib import ExitStack

    ctx = ExitStack()
    with ctx:
        const = ctx.enter_context(tc.tile_pool(name="const", bufs=1))
        big = ctx.enter_context(tc.tile_pool(name="big", bufs=1))
        tiny = ctx.enter_context(tc.tile_pool(name="tiny", bufs=1))

        # ---- big input DMAs first (sync/HWDGE ring), chunked so dependents
        # ---- can start before the full tensor lands ----
        xT = big.tile([65, S], BF16)  # row 64 = ones (feeds v's l-column)
        for i in range(4):
            nc.sync.dma_start(out=xT[0:64, 1024 * i:1024 * (i + 1)],
                              in_=xT_d[:, 1024 * i:1024 * (i + 1)])
        nc.gpsimd.memset(xT[64:65, :], 1.0)
        xq_sb = big.tile([128, NQ, 64], F32)
        nc.sync.dma_start(out=xq_sb, in_=x_q.rearrange("(m p) c -> p m c", p=128))

        # ---- params via the scalar-engine HWDGE ring (parallel with sync) ----
        wq_aug = const.tile([65, 64], F32)   # [Wq ; bq]
        nc.scalar.dma_start(out=wq_aug[0:64, :], in_=wq_d)
        nc.scalar.dma_start(out=wq_aug[64:65, :], in_=bq_d.rearrange("(o c) -> o c", o=1))
        wk_sb = const.tile([64, 64], F32)
        nc.scalar.dma_start(out=wk_sb, in_=wk_d)
        wv_aug = const.tile([65, 65], F32)   # [Wv ; bv] plus e64 column
        nc.scalar.dma_start(out=wv_aug[0:64, 0:64], in_=wv_d)
        nc.scalar.dma_start(out=wv_aug[64:65, 0:64], in_=bv_d.rearrange("(o c) -> o c", o=1))
        nc.gpsimd.memset(wv_aug[0:64, 64:65], 0.0)
        nc.gpsimd.memset(wv_aug[64:65, 64:65], 1.0)
        # wo_aug = [wo ; bvo] plus e64 column that passes l through. Row 64
        # multiplies the l-row of the accumulator, so after the division by l
        # it contributes the constant row bvo = bv_total @ wo - this is how the
        # v-bias is applied without ever materializing it per-position.
        wo_aug = const.tile([65, 65], BF16)
        nc.gpsimd.dma_start(out=wo_aug[0:64, 0:64], in_=wo_d)  # SWDGE casts f32->bf16
        nc.gpsimd.memset(wo_aug[0:64, 64:65], 0.0)
        nc.gpsimd.memset(wo_aug[64:65, 64:65], 1.0)
        wo_sb = const.tile([64, 64], F32)
        nc.scalar.dma_start(out=wo_sb, in_=wo_d)
        gamma_col = const.tile([64, 1], F32)
        nc.scalar.dma_start(out=gamma_col, in_=gamma.rearrange("(c o) -> c o", o=1))
        beta_col = const.tile([64, 1], F32)
        nc.scalar.dma_start(out=beta_col, in_=beta.rearrange("(c o) -> c o", o=1))
        bo_bcast = const.tile([128, 64], F32)
        nc.scalar.dma_start(out=bo_bcast, in_=bo_d.rearrange("(o c) -> o c", o=1).to_broadcast([128, 64]))

        zbias = const.tile([128, 1], F32)
        nc.gpsimd.memset(zbias, 0.0)
        eps32 = const.tile([32, 1], F32)
        nc.gpsimd.memset(eps32, EPS)
        # Preload the sqrt ACT table set while waiting on input DMAs so the
        # stats chain doesn't eat the ~1.3us table load.
        scratch1 = const.tile([1, 1], F32)
        nc.scalar.activation(scratch1, eps32[0:1, :], AF.Sqrt, bias=0.0, scale=1.0)

        # pair matrices: p64h[c,g] = 0.5 iff c//2 == g ; p32x64[g,c] = 1 iff c//2 == g
        p64h = const.tile([64, 32], F32)
        nc.gpsimd.memset(p64h, 0.5)
        nc.gpsimd.affine_select(out=p64h, in_=p64h, compare_op=ALU.is_ge,
                                fill=0.0, base=0, pattern=[[-2, 32]],
                                channel_multiplier=1)
        nc.gpsimd.affine_select(out=p64h, in_=p64h, compare_op=ALU.is_ge,
                                fill=0.0, base=1, pattern=[[2, 32]],
                                channel_multiplier=-1)
        p32x64 = const.tile([32, 64], F32)
        nc.gpsimd.memset(p32x64, 1.0)
        nc.gpsimd.affine_select(out=p32x64, in_=p32x64, compare_op=ALU.is_ge,
                                fill=0.0, base=0, pattern=[[1, 64]],
                                channel_multiplier=-2)
        nc.gpsimd.affine_select(out=p32x64, in_=p32x64, compare_op=ALU.is_ge,
                                fill=0.0, base=1, pattern=[[-1, 64]],
                                channel_multiplier=2)

        # ---- PSUM pools (8 banks: 2x2 + 2 + 2) ----
        # 8 PSUM banks: st 3x[128,1024] = 6, ot 2x[65,512] = 2. Small prep
        # matmuls borrow ot slots; epilogue/v-proj matmuls borrow st slots.
        st_ps = ctx.enter_context(tc.tile_pool(name="st_ps", bufs=3, space="PSUM"))
        ot_ps = ctx.enter_context(tc.tile_pool(name="ot_ps", bufs=2, space="PSUM"))

        # ---- GroupNorm stats on DVE: per-channel mean/var over all 4096 ----
        bstats = tiny.tile([64, 8, 6], F32)
        for i in range(8):
            nc.vector.bn_stats(bstats[:, i, :], xT[0:64, 512 * i:512 * (i + 1)])
        mv = tiny.tile([64, 2], F32)
        nc.vector.bn_aggr(mv, bstats)
        packed64 = tiny.tile([64, 2], F32)        # [mean_c, E[x^2]_c]
        nc.vector.tensor_copy(packed64[:, 0:1], mv[:, 0:1])
        nc.vector.tensor_mul(packed64[:, 1:2], mv[:, 0:1], mv[:, 0:1])
        nc.vector.tensor_add(packed64[:, 1:2], packed64[:, 1:2], mv[:, 1:2])
        gpair = ot_ps.tile([32, 2], F32, tag="ot")  # group [mean, E[x^2]]
        nc.tensor.matmul(gpair, lhsT=p64h, rhs=packed64)
        gm = tiny.tile([32, 2], F32)
        nc.vector.tensor_copy(gm, gpair)
        var = tiny.tile([32, 1], F32)
        nc.vector.tensor_mul(var, gm[:, 0:1], gm[:, 0:1])
        nc.vector.tensor_sub(var, gm[:, 1:2], var)
        std = tiny.tile([32, 1], F32)
        nc.scalar.activation(std, var, AF.Sqrt, bias=eps32, scale=1.0)
        # Swap in the exp table set now, before the main loop needs it.
        scratch2 = const.tile([1, 1], F32)
        nc.scalar.activation(scratch2, eps32[0:1, :], AF.Exp, bias=0.0, scale=1.0)
        rstd = tiny.tile([32, 1], F32)
        nc.vector.reciprocal(rstd, std)
        packed32 = tiny.tile([32, 2], F32)        # [rstd_g | mean_g]
        nc.vector.tensor_copy(packed32[:, 0:1], rstd)
        nc.vector.tensor_copy(packed32[:, 1:2], gm[:, 0:1])
        chan = ot_ps.tile([64, 2], F32, tag="ot")  # expand groups->channels
        nc.tensor.matmul(chan, lhsT=p32x64, rhs=packed32)
        scale_col = tiny.tile([64, 1], F32)       # rstd_g * gamma_c
        nc.vector.tensor_mul(scale_col, chan[:, 0:1], gamma_col)
        gnbias = tiny.tile([65, 1], F32)          # beta - mean*scale, aug 1
        nc.vector.tensor_mul(gnbias[0:64, :], chan[:, 1:2], scale_col)
        nc.vector.tensor_sub(gnbias[0:64, :], beta_col, gnbias[0:64, :])
        nc.gpsimd.memset(gnbias[64:65, :], 1.0)

        # ---- fold GN into projection weights ----
        wq_sc = tiny.tile([64, 64], BF16)
        nc.vector.tensor_scalar_mul(wq_sc, wq_aug[0:64, :], scale_col)
        wk_sc = tiny.tile([64, 64], BF16)
        nc.vector.tensor_scalar_mul(wk_sc, wk_sb, scale_col)
        # wv_sc rows 0:64 = scaled Wv (no bias); row 64 = e64 so that the ones
        # row of xT produces v's l-column.
        wv_sc = tiny.tile([65, 65], BF16)
        nc.vector.tensor_scalar_mul(wv_sc[0:64, 0:64], wv_aug[0:64, 0:64], scale_col)
        nc.gpsimd.memset(wv_sc[0:64, 64:65], 0.0)
        nc.gpsimd.memset(wv_sc[64:65, 0:64], 0.0)
        nc.gpsimd.memset(wv_sc[64:65, 64:65], 1.0)

        bqp = ot_ps.tile([64, 1], F32, tag="ot")  # total q bias (column)
        nc.tensor.matmul(bqp, lhsT=wq_aug, rhs=gnbias)
        bq_col = tiny.tile([64, 1], F32)
        nc.vector.tensor_copy(bq_col, bqp)
        # bvo row for wo_aug: bvo = (gnbias@Wv + bv) @ wo, bounced through HBM
        # to land on partition 64 (engines are lane-locked; DMA is not). This
        # only gates the first output projection, well off the critical path.
        bvcp = ot_ps.tile([65, 1], F32, tag="ot")
        nc.tensor.matmul(bvcp, lhsT=wv_aug, rhs=gnbias)
        bv_col = tiny.tile([64, 1], F32)
        nc.vector.tensor_copy(bv_col, bvcp[0:64, :])
        bvop = ot_ps.tile([1, 64], F32, tag="ot")
        nc.tensor.matmul(bvop, lhsT=bv_col, rhs=wo_sb)
        bvo_row = tiny.tile([1, 64], F32)
        nc.vector.tensor_copy(bvo_row, bvop)
        bvo_stage = nc.dram_tensor("bvo_stage", [64], F32).ap()
        nc.sync.dma_start(out=bvo_stage.rearrange("(o c) -> o c", o=1), in_=bvo_row)
        nc.gpsimd.dma_start(out=wo_aug[64:65, 0:64],
                            in_=bvo_stage.rearrange("(o c) -> o c", o=1))

        # ---- projections; kT/qT mirrored to partitions 64:127 for row-tiling ----
        kT = big.tile([128, S], BF16)
        qT = big.tile([128, SQ], BF16)
        for j in range(8):
            kp = st_ps.tile([64, 512], F32, tag="st")
            nc.tensor.matmul(kp, lhsT=wk_sc, rhs=xT[0:64, 512 * j:512 * (j + 1)])
            nc.scalar.copy(out=kT[0:64, 512 * j:512 * (j + 1)], in_=kp)
            nc.sync.dma_start(out=kT[64:128, 512 * j:512 * (j + 1)],
                              in_=kT[0:64, 512 * j:512 * (j + 1)])
            if j < N_STRIPE:
                qp = st_ps.tile([64, 512], F32, tag="st")
                nc.tensor.matmul(qp, lhsT=wq_sc, rhs=xT[0:64, 512 * j:512 * (j + 1)])
                nc.vector.tensor_scalar_add(qT[0:64, 512 * j:512 * (j + 1)], qp, bq_col)
                nc.sync.dma_start(out=qT[64:128, 512 * j:512 * (j + 1)],
                                  in_=qT[0:64, 512 * j:512 * (j + 1)])

        v_sb = []
        for n in range(N_CHUNK):
            vp = st_ps.tile([128, 65], F32, tag="st")
            nc.tensor.matmul(vp, lhsT=xT[:, 128 * n:128 * (n + 1)], rhs=wv_sc)
            vt = big.tile([128, 65], BF16, tag="v", bufs=N_CHUNK)
            nc.vector.tensor_copy(vt, vp)
            v_sb.append(vt)

        # ---- residual base: x + bo ----
        xb_sb = big.tile([128, NQ, 64], F32)
        for m in range(NQ):
            nc.vector.tensor_add(xb_sb[:, m, :], xq_sb[:, m, :], bo_bcast)

        # ---- main attention loop ----
        # Pairs of key chunks: the two K=64 score matmuls run concurrently on
        # the two row-halves of the PE array into the two banks of one PSUM
        # tile; exp covers both in one ACT instruction. att@v for each chunk
        # is split into two K=64 halves (lo/hi array rows) accumulating into
        # separate PSUM banks, summed once per stripe. All att@v work is
        # emitted one pair behind its exp so it never stalls the PE queue.
        p_pool = ctx.enter_context(tc.tile_pool(name="p_pool", bufs=4))
        ep_pool = ctx.enter_context(tc.tile_pool(name="ep_pool", bufs=3))
        N_PAIR = N_CHUNK // 2

        def emit_o(io, ot_lo, ot_hi, pt):
            nc.tensor.matmul(ot_lo, lhsT=v_sb[io][0:64, :], rhs=pt[0:64, :],
                             tile_position=(0, 0),
                             start=(io == 0), stop=(io == N_CHUNK - 1))
            nc.tensor.matmul(ot_hi, lhsT=v_sb[io][64:128, :], rhs=pt[64:128, :],
                             tile_position=(64, 0),
                             start=(io == 0), stop=(io == N_CHUNK - 1))

        def make_epilogue(j, ot_sb):
            def epi():
                for m in range(4):
                    op = st_ps.tile([128, 65], F32, tag="st")
                    nc.tensor.matmul(op, lhsT=ot_sb[:, 128 * m:128 * (m + 1)],
                                     rhs=wo_aug)
                    rl = ep_pool.tile([128, 1], F32, tag="rl")
                    nc.vector.reciprocal(rl, op[:, 64:65])
                    res = ep_pool.tile([128, 64], F32, tag="res")
                    nc.vector.scalar_tensor_tensor(out=res, in0=op[:, 0:64],
                                                   scalar=rl,
                                                   in1=xb_sb[:, 4 * j + m, :],
                                                   op0=ALU.mult, op1=ALU.add)
                    base = 512 * j + 128 * m
                    nc.sync.dma_start(out=out_d[base:base + 128, :], in_=res)
            return epi

        pending_epilogue = None
        for j in range(N_STRIPE):
            ot_lo = ot_ps.tile([65, 512], F32, tag="ot")
            ot_hi = ot_ps.tile([65, 512], F32, tag="ot")
            pts = {}
            for p in range(N_PAIR + 1):
                if p < N_PAIR:
                    i0, i1 = 2 * p, 2 * p + 1
                    st2 = st_ps.tile([128, 1024], F32, tag="st")
                    nc.tensor.matmul(st2[:, 0:512],
                                     lhsT=kT[0:64, 128 * i0:128 * (i0 + 1)],
                                     rhs=qT[0:64, 512 * j:512 * (j + 1)],
                                     tile_position=(0, 0))
                    nc.tensor.matmul(st2[:, 512:1024],
                                     lhsT=kT[64:128, 128 * i1:128 * (i1 + 1)],
                                     rhs=qT[64:128, 512 * j:512 * (j + 1)],
                                     tile_position=(64, 0))
                    pt = p_pool.tile([128, 1024], BF16, tag="p")
                    nc.scalar.activation(pt, st2, AF.Exp, bias=zbias, scale=SCALE)
                    pts[p] = pt
                if p == 3 and pending_epilogue is not None:
                    pending_epilogue()
                    pending_epilogue = None
                po = p - 1
                if po >= 0:
                    pt = pts.pop(po)
                    emit_o(2 * po, ot_lo, ot_hi, pt[:, 0:512])
                    emit_o(2 * po + 1, ot_lo, ot_hi, pt[:, 512:1024])
            # merge halves (+ l row); DVE may read only one PSUM input per op
            ot_sb = ep_pool.tile([65, 512], BF16, bufs=2, tag="ot_sb")
            nc.vector.tensor_copy(ot_sb, ot_lo)
            nc.vector.tensor_add(ot_sb, ot_sb, ot_hi)
            pending_epilogue = make_epilogue(j, ot_sb)
        pending_epilogue()


_NC_CACHE = {}


def _get_nc():
    if "nc" not in _NC_CACHE:
        _NC_CACHE["nc"] = build_kernel()
    return _NC_CACHE["nc"]


def build_in_maps(x, gamma, beta, wq, bq, wk, wv, bv, wo, bo):
    """Per-core NEFF input dicts plus (batch, rows) scatter info per core."""
    x = np.asarray(x, dtype=np.float32)
    shared = {
        "gamma": np.asarray(gamma, np.float32),
        "beta": np.asarray(beta, np.float32),
        "wq": np.asarray(wq, np.float32), "bq": np.asarray(bq, np.float32),
        "wk": np.asarray(wk, np.float32),
        "wv": np.asarray(wv, np.float32), "bv": np.asarray(bv, np.float32),
        "wo": np.asarray(wo, np.float32), "bo": np.asarray(bo, np.float32),
    }
    xf = x.reshape(B, S, C)
    in_maps = []
    scatter = []
    for core in range(8):
        b, h = core // 2, core % 2
        own = slice(h * SQ, (h + 1) * SQ)
        other = slice((1 - h) * SQ, (2 - h) * SQ)
        x_local = np.concatenate([xf[b][own], xf[b][other]], axis=0)
        in_maps.append({
            "xT": np.ascontiguousarray(x_local.T).astype(ml_dtypes.bfloat16),
            "x_q": np.ascontiguousarray(x_local[:SQ]),
            **shared,
        })
        scatter.append((b, np.arange(h * SQ, (h + 1) * SQ)))
    return in_maps, scatter


def _run(in_maps, scatter, **spmd_kwargs):
    nc = _get_nc()
    res = run_bass_kernel_spmd(nc, in_maps, core_ids=list(range(8)),
                               **spmd_kwargs)
    out = np.empty((B, S, C), np.float32)
    for core in range(8):
        b, rows = scatter[core]
        out[b][rows] = res.results[core]["out"]
    return out.reshape(B, H, W, C), res


def kernel(x, gamma, beta, wq, bq, wk, bk, wv, bv, wo, bo):
    # bk is provably a no-op: it shifts each query's scores by the constant
    # bk.q which softmax cancels, so it is not shipped to the device.
    in_maps, scatter = build_in_maps(x, gamma, beta, wq, bq, wk, wv, bv, wo, bo)
    out, _ = _run(in_maps, scatter)
    return out


# revision 18
# speedup vs baseline: 1.8703x; 1.0446x over previous
"""Trainium2 Bass kernel for nn_AttentionBlock (B=4, H=W=64, C=64, GROUPS=32).

Math (reference):
    hn = GroupNorm(x; gamma, beta, 32 groups, eps=1e-3)
    q = hn@wq+bq ; k = hn@wk+bk ; v = hn@wv+bv
    att = softmax(q k^T / 8) over the 4096 spatial positions
    out = x + (att @ v) @ wo + bo

Sharding: data-parallel, 2 cores per batch image, each core owns 2048 of the
4096 queries but holds the full key/value set for its batch. No collectives.

Per-core pipeline (fully fused on one NeuronCore):
  - xT [C=64, S=4096] arrives pre-transposed in bf16 (host does the cheap
    numpy transpose+cast), so channel-contraction matmuls need no on-chip
    transposes. x_q keeps the core's own query rows in fp32 for the residual.
  - GroupNorm stats via bn_stats/bn_aggr per channel on DVE, then tiny 0/1
    matmuls pair-combine channels into groups and expand back. The GN affine
    folds into the projection weights: W~ = diag(scale_c)@W, b~ = gnbias@W + b.
  - k-bias is dropped: it shifts each query's scores by a constant, which
    softmax cancels exactly.
  - Scores are computed transposed, ST[t, s] (keys on partitions), so exp(ST)
    feeds the att@v matmul directly as the moving operand - the attention
    matrix is never transposed. Score matmuls have K=64, so two key-chunks run
    CONCURRENTLY on the two halves of the PE array (row-tiling), with kT/qT
    mirrored onto partitions 64:127 by one SBUF-to-SBUF DMA each.
  - Softmax is max-free: |score| <= ~3 for unit-normal inputs so exp cannot
    overflow, and softmax(x) == softmax(x - max) exactly.
  - exp() runs one ACT instruction per chunk-pair over a 2-bank PSUM tile to
    amortize the ~352-cycle activation pipeline latency.
  - v gets an appended ones-column so att@v also accumulates the softmax
    denominator l[s]. att@v is split into two K=64 halves accumulating into
    two PSUM banks (summed by one DVE add at stripe end): the halves run on
    opposite array halves, letting LDWEIGHTS overlap in-flight matmuls.
  - The output projection runs on the unnormalized accumulator ((O/l)@wo ==
    (O@wo)/l), with an extra wo column passing l through; one reciprocal +
    fused multiply-add applies softmax normalization, residual and bo.
"""

import numpy as np
import ml_dtypes

import concourse.bass as bass
import concourse.tile as tile
from concourse import bacc, mybir
from concourse.bass_utils import run_bass_kernel_spmd

F32 = mybir.dt.float32
BF16 = mybir.dt.bfloat16
AF = mybir.ActivationFunctionType
ALU = mybir.AluOpType

B, H, W, C = 4, 64, 64, 64
S = H * W            # 4096 spatial positions per image
SQ = S // 2          # 2048 queries per core
EPS = 1e-3
N_CHUNK = S // 128   # 32 key chunks
NQ = SQ // 128       # 16 query chunks
N_STRIPE = SQ // 512  # 4 query stripes
SCALE = float(C) ** -0.5  # 0.125


def build_kernel():
    nc = bacc.Bacc("TRN2", target_bir_lowering=False, debug=False)

    xT_d = nc.dram_tensor("xT", [C, S], BF16, kind="ExternalInput")
    x_q = nc.dram_tensor("x_q", [SQ, C], F32, kind="ExternalInput")
    gamma = nc.dram_tensor("gamma", [C], F32, kind="ExternalInput")
    beta = nc.dram_tensor("beta", [C], F32, kind="ExternalInput")
    wq_d = nc.dram_tensor("wq", [C, C], F32, kind="ExternalInput")
    bq_d = nc.dram_tensor("bq", [C], F32, kind="ExternalInput")
    wk_d = nc.dram_tensor("wk", [C, C], F32, kind="ExternalInput")
    wv_d = nc.dram_tensor("wv", [C, C], F32, kind="ExternalInput")
    bv_d = nc.dram_tensor("bv", [C], F32, kind="ExternalInput")
    wo_d = nc.dram_tensor("wo", [C, C], F32, kind="ExternalInput")
    bo_d = nc.dram_tensor("bo", [C], F32, kind="ExternalInput")
    out_d = nc.dram_tensor("out", [SQ, C], F32, kind="ExternalOutput")

    with tile.TileContext(nc) as tc:
        _emit(nc, tc, xT_d.ap(), x_q.ap(), gamma.ap(), beta.ap(), wq_d.ap(),
              bq_d.ap(), wk_d.ap(), wv_d.ap(), bv_d.ap(), wo_d.ap(), bo_d.ap(),
              out_d.ap())
    nc.compile()
    return nc


def _emit(nc, tc, xT_d, x_q, gamma, beta, wq_d, bq_d, wk_d, wv_d, bv_d, wo_d,
          bo_d, out_d):
    from contextlib import ExitStack

    ctx = ExitStack()
    with ctx:
        const = ctx.enter_context(tc.tile_pool(name="const", bufs=1))
        big = ctx.enter_context(tc.tile_pool(name="big", bufs=1))
        tiny = ctx.enter_context(tc.tile_pool(name="tiny", bufs=1))

        # ---- big input DMAs first (sync/HWDGE ring), chunked so dependents
        # ---- can start early; partitions 64:127 mirror 0:63 for row-tiling ----
        xT = big.tile([128, S], BF16)
        for i in range(4):
            nc.sync.dma_start(out=xT[0:64, 1024 * i:1024 * (i + 1)],
                              in_=xT_d[:, 1024 * i:1024 * (i + 1)])
            nc.sync.dma_start(out=xT[64:128, 1024 * i:1024 * (i + 1)],
                              in_=xT[0:64, 1024 * i:1024 * (i + 1)])

        # ---- params via the scalar-engine HWDGE ring (parallel with sync) ----
        wq_aug = const.tile([65, 64], F32)   # [Wq ; bq]
        nc.scalar.dma_start(out=wq_aug[0:64, :], in_=wq_d)
        nc.scalar.dma_start(out=wq_aug[64:65, :], in_=bq_d.rearrange("(o c) -> o c", o=1))
        wk_sb = const.tile([64, 64], F32)
        nc.scalar.dma_start(out=wk_sb, in_=wk_d)
        wv_aug = const.tile([65, 65], F32)   # [Wv ; bv] plus e64 column
        nc.scalar.dma_start(out=wv_aug[0:64, 0:64], in_=wv_d)
        nc.scalar.dma_start(out=wv_aug[64:65, 0:64], in_=bv_d.rearrange("(o c) -> o c", o=1))
        nc.gpsimd.memset(wv_aug[0:64, 64:65], 0.0)
        nc.gpsimd.memset(wv_aug[64:65, 64:65], 1.0)
        # wo_aug = [wo ; bvo] plus e64 column that passes l through. Row 64
        # multiplies the l-row of the accumulator, so after the division by l
        # it contributes the constant row bvo = bv_total @ wo - this is how the
        # v-bias is applied without ever materializing it per-position.
        wo_aug = const.tile([65, 65], BF16)
        nc.gpsimd.dma_start(out=wo_aug[0:64, 0:64], in_=wo_d)  # SWDGE casts f32->bf16
        nc.gpsimd.memset(wo_aug[0:64, 64:65], 0.0)
        nc.gpsimd.memset(wo_aug[64:65, 64:65], 1.0)
        wo_sb = const.tile([64, 64], F32)
        nc.scalar.dma_start(out=wo_sb, in_=wo_d)
        gamma_col = const.tile([64, 1], F32)
        nc.scalar.dma_start(out=gamma_col, in_=gamma.rearrange("(c o) -> c o", o=1))
        beta_col = const.tile([64, 1], F32)
        nc.scalar.dma_start(out=beta_col, in_=beta.rearrange("(c o) -> c o", o=1))
        bo_bcast = const.tile([128, 64], F32)
        nc.scalar.dma_start(out=bo_bcast, in_=bo_d.rearrange("(o c) -> o c", o=1).to_broadcast([128, 64]))

        zbias = const.tile([128, 1], F32)
        nc.gpsimd.memset(zbias, 0.0)
        eps32 = const.tile([32, 1], F32)
        nc.gpsimd.memset(eps32, EPS)
        # Preload the sqrt ACT table set while waiting on input DMAs so the
        # stats chain doesn't eat the ~1.3us table load.
        scratch1 = const.tile([1, 1], F32)
        nc.scalar.activation(scratch1, eps32[0:1, :], AF.Sqrt, bias=0.0, scale=1.0)

        # pair matrices: p64h[c,g] = 0.5 iff c//2 == g ; p32x64[g,c] = 1 iff c//2 == g
        p64h = const.tile([64, 32], F32)
        nc.gpsimd.memset(p64h, 0.5)
        nc.gpsimd.affine_select(out=p64h, in_=p64h, compare_op=ALU.is_ge,
                                fill=0.0, base=0, pattern=[[-2, 32]],
                                channel_multiplier=1)
        nc.gpsimd.affine_select(out=p64h, in_=p64h, compare_op=ALU.is_ge,
                                fill=0.0, base=1, pattern=[[2, 32]],
                                channel_multiplier=-1)
        p32x64 = const.tile([32, 64], F32)
        nc.gpsimd.memset(p32x64, 1.0)
        nc.gpsimd.affine_select(out=p32x64, in_=p32x64, compare_op=ALU.is_ge,
                                fill=0.0, base=0, pattern=[[1, 64]],
                                channel_multiplier=-2)
        nc.gpsimd.affine_select(out=p32x64, in_=p32x64, compare_op=ALU.is_ge,
                                fill=0.0, base=1, pattern=[[-1, 64]],
                                channel_multiplier=2)

        # ---- PSUM pools (8 banks: st 2x[128,1024] = 4, aux 4x one-bank) ----
        st_ps = ctx.enter_context(tc.tile_pool(name="st_ps", bufs=2, space="PSUM"))
        aux_ps = ctx.enter_context(tc.tile_pool(name="aux_ps", bufs=4, space="PSUM"))

        # ---- GroupNorm stats on DVE: per-channel mean/var over all 4096 ----
        bstats = tiny.tile([64, 8, 6], F32)
        for i in range(8):
            nc.vector.bn_stats(bstats[:, i, :], xT[0:64, 512 * i:512 * (i + 1)])
        mv = tiny.tile([64, 2], F32)
        nc.vector.bn_aggr(mv, bstats)
        packed64 = tiny.tile([64, 2], F32)        # [mean_c, E[x^2]_c]
        nc.vector.tensor_copy(packed64[:, 0:1], mv[:, 0:1])
        nc.vector.tensor_mul(packed64[:, 1:2], mv[:, 0:1], mv[:, 0:1])
        nc.vector.tensor_add(packed64[:, 1:2], packed64[:, 1:2], mv[:, 1:2])
        gpair = aux_ps.tile([32, 2], F32, tag="aux")  # group [mean, E[x^2]]
        nc.tensor.matmul(gpair, lhsT=p64h, rhs=packed64)
        gm = tiny.tile([32, 2], F32)
        nc.vector.tensor_copy(gm, gpair)
        var = tiny.tile([32, 1], F32)
        nc.vector.tensor_mul(var, gm[:, 0:1], gm[:, 0:1])
        nc.vector.tensor_sub(var, gm[:, 1:2], var)
        std = tiny.tile([32, 1], F32)
        nc.scalar.activation(std, var, AF.Sqrt, bias=eps32, scale=1.0)
        # Swap in the exp table set now, before the main loop needs it.
        scratch2 = const.tile([1, 1], F32)
        nc.scalar.activation(scratch2, eps32[0:1, :], AF.Exp, bias=0.0, scale=1.0)
        rstd = tiny.tile([32, 1], F32)
        nc.vector.reciprocal(rstd, std)
        packed32 = tiny.tile([32, 2], F32)        # [rstd_g | mean_g]
        nc.vector.tensor_copy(packed32[:, 0:1], rstd)
        nc.vector.tensor_copy(packed32[:, 1:2], gm[:, 0:1])
        chan = aux_ps.tile([64, 2], F32, tag="aux")  # expand groups->channels
        nc.tensor.matmul(chan, lhsT=p32x64, rhs=packed32)
        scale_col = tiny.tile([64, 1], F32)       # rstd_g * gamma_c
        nc.vector.tensor_mul(scale_col, chan[:, 0:1], gamma_col)
        gnbias = tiny.tile([65, 1], F32)          # beta - mean*scale, aug 1
        nc.vector.tensor_mul(gnbias[0:64, :], chan[:, 1:2], scale_col)
        nc.vector.tensor_sub(gnbias[0:64, :], beta_col, gnbias[0:64, :])
        nc.gpsimd.memset(gnbias[64:65, :], 1.0)

        # ---- fold GN into projection weights (mirrored for row-tiling) ----
        wq_sc = tiny.tile([128, 64], BF16)
        nc.vector.tensor_scalar_mul(wq_sc[0:64, :], wq_aug[0:64, :], scale_col)
        nc.sync.dma_start(out=wq_sc[64:128, :], in_=wq_sc[0:64, :])
        wk_sc = tiny.tile([128, 64], BF16)
        nc.vector.tensor_scalar_mul(wk_sc[0:64, :], wk_sb, scale_col)
        nc.sync.dma_start(out=wk_sc[64:128, :], in_=wk_sc[0:64, :])
        wv_sc = tiny.tile([128, 64], BF16)
        nc.vector.tensor_scalar_mul(wv_sc[0:64, :], wv_aug[0:64, 0:64], scale_col)
        nc.sync.dma_start(out=wv_sc[64:128, :], in_=wv_sc[0:64, :])

        bqp = aux_ps.tile([64, 1], F32, tag="aux")  # total q bias (column)
        nc.tensor.matmul(bqp, lhsT=wq_aug, rhs=gnbias)
        bq_col = tiny.tile([64, 1], F32)
        nc.vector.tensor_copy(bq_col, bqp)
        # bvo row for wo_aug: bvo = (gnbias@Wv + bv) @ wo, bounced through HBM
        # to land on partition 64 (engines are lane-locked; DMA is not). This
        # only gates the first output projection, well off the critical path.
        bvcp = aux_ps.tile([65, 1], F32, tag="aux")
        nc.tensor.matmul(bvcp, lhsT=wv_aug, rhs=gnbias)
        bv_col = tiny.tile([64, 1], F32)
        nc.vector.tensor_copy(bv_col, bvcp[0:64, :])
        bvop = aux_ps.tile([1, 64], F32, tag="aux")
        nc.tensor.matmul(bvop, lhsT=bv_col, rhs=wo_sb)
        bvo_row = tiny.tile([1, 64], F32)
        nc.vector.tensor_copy(bvo_row, bvop)
        bvo_stage = nc.dram_tensor("bvo_stage", [64], F32).ap()
        nc.sync.dma_start(out=bvo_stage.rearrange("(o c) -> o c", o=1), in_=bvo_row)
        nc.gpsimd.dma_start(out=wo_aug[64:65, 0:64],
                            in_=bvo_stage.rearrange("(o c) -> o c", o=1))

        # ---- projections: row-tiled pairs (even chunk on array rows 0:63,
        # ---- odd chunk on rows 64:127); kT/qT mirrored via per-chunk DMAs ----
        kT = big.tile([128, S], BF16)
        qT = big.tile([128, SQ], BF16)
        for j in range(4):
            kpa = aux_ps.tile([64, 512], F32, tag="aux")
            kpb = aux_ps.tile([64, 512], F32, tag="aux")
            ja, jb = 2 * j, 2 * j + 1
            nc.tensor.matmul(kpa, lhsT=wk_sc[0:64, :],
                             rhs=xT[0:64, 512 * ja:512 * (ja + 1)],
                             tile_position=(0, 0))
            nc.tensor.matmul(kpb, lhsT=wk_sc[64:128, :],
                             rhs=xT[64:128, 512 * jb:512 * (jb + 1)],
                             tile_position=(64, 0))
            nc.scalar.copy(out=kT[0:64, 512 * ja:512 * (ja + 1)], in_=kpa)
            nc.scalar.copy(out=kT[0:64, 512 * jb:512 * (jb + 1)], in_=kpb)
            nc.sync.dma_start(out=kT[64:128, 1024 * j:1024 * (j + 1)],
                              in_=kT[0:64, 1024 * j:1024 * (j + 1)])
            if j < 2:
                qpa = aux_ps.tile([64, 512], F32, tag="aux")
                qpb = aux_ps.tile([64, 512], F32, tag="aux")
                nc.tensor.matmul(qpa, lhsT=wq_sc[0:64, :],
                                 rhs=xT[0:64, 512 * ja:512 * (ja + 1)],
                                 tile_position=(0, 0))
                nc.tensor.matmul(qpb, lhsT=wq_sc[64:128, :],
                                 rhs=xT[64:128, 512 * jb:512 * (jb + 1)],
                                 tile_position=(64, 0))
                nc.vector.tensor_scalar_add(qT[0:64, 512 * ja:512 * (ja + 1)], qpa, bq_col)
                nc.vector.tensor_scalar_add(qT[0:64, 512 * jb:512 * (jb + 1)], qpb, bq_col)
                nc.sync.dma_start(out=qT[64:128, 1024 * j:1024 * (j + 1)],
                                  in_=qT[0:64, 1024 * j:1024 * (j + 1)])

        # v in natural [t, c] layout, one big tile; col 64 = ones (one memset)
        v_big = big.tile([128, N_CHUNK, 65], BF16)
        nc.gpsimd.memset(v_big[:, :, 64:65], 1.0)
        for p in range(N_CHUNK // 2):
            i0, i1 = 2 * p, 2 * p + 1
            vpa = aux_ps.tile([128, 64], F32, tag="aux")
            vpb = aux_ps.tile([128, 64], F32, tag="aux")
            nc.tensor.matmul(vpa, lhsT=xT[0:64, 128 * i0:128 * (i0 + 1)],
                             rhs=wv_sc[0:64, :], tile_position=(0, 0))
            nc.tensor.matmul(vpb, lhsT=xT[64:128, 128 * i1:128 * (i1 + 1)],
                             rhs=wv_sc[64:128, :], tile_position=(64, 0))
            nc.vector.tensor_copy(v_big[:, i0, 0:64], vpa)
            nc.vector.tensor_copy(v_big[:, i1, 0:64], vpb)

        # ---- residual base: x + bo (needed only by epilogues) ----
        xq_sb = big.tile([128, NQ, 64], F32)
        nc.sync.dma_start(out=xq_sb, in_=x_q.rearrange("(m p) c -> p m c", p=128))
        xb_sb = big.tile([128, NQ, 64], F32)
        for m in range(NQ):
            nc.vector.tensor_add(xb_sb[:, m, :], xq_sb[:, m, :], bo_bcast)

        # ---- main attention loop ----
        # Pairs of key chunks: the two K=64 score matmuls run concurrently on
        # the two row-halves of the PE array into the two banks of one PSUM
        # tile; exp covers both in one ACT instruction. att@v for each chunk
        # is split into two K=64 halves (lo/hi array rows) accumulating into
        # separate PSUM banks, summed once per stripe. All att@v work is
        # emitted one pair behind its exp so it never stalls the PE queue.
        p_pool = ctx.enter_context(tc.tile_pool(name="p_pool", bufs=4))
        ep_pool = ctx.enter_context(tc.tile_pool(name="ep_pool", bufs=3))
        N_PAIR = N_CHUNK // 2

        def emit_o(io, ot_lo, ot_hi, pt):
            nc.tensor.matmul(ot_lo, lhsT=v_big[0:64, io, :], rhs=pt[0:64, :],
                             tile_position=(0, 0),
                             start=(io == 0), stop=(io == N_CHUNK - 1))
            nc.tensor.matmul(ot_hi, lhsT=v_big[64:128, io, :], rhs=pt[64:128, :],
                             tile_position=(64, 0),
                             start=(io == 0), stop=(io == N_CHUNK - 1))

        def make_epilogue(j, ot_sb):
            def epi():
                res = ep_pool.tile([128, 4, 64], F32, tag="res", bufs=2)
                for m in range(4):
                    op = aux_ps.tile([128, 65], F32, tag="aux")
                    nc.tensor.matmul(op, lhsT=ot_sb[:, 128 * m:128 * (m + 1)],
                                     rhs=wo_aug)
                    rl = ep_pool.tile([128, 1], F32, tag="rl")
                    nc.vector.reciprocal(rl, op[:, 64:65])
                    nc.vector.scalar_tensor_tensor(out=res[:, m, :],
                                                   in0=op[:, 0:64],
                                                   scalar=rl,
                                                   in1=xb_sb[:, 4 * j + m, :],
                                                   op0=ALU.mult, op1=ALU.add)
                nc.sync.dma_start(
                    out=out_d[512 * j:512 * (j + 1), :].rearrange("(m p) c -> p m c", p=128),
                    in_=res)
            return epi

        pending_epilogue = None
        for j in range(N_STRIPE):
            ot_lo = aux_ps.tile([65, 512], F32, tag="aux")
            ot_hi = aux_ps.tile([65, 512], F32, tag="aux")
            pts = {}
            for p in range(N_PAIR + 1):
                if p < N_PAIR:
                    i0, i1 = 2 * p, 2 * p + 1
                    st2 = st_ps.tile([128, 1024], F32, tag="st")
                    nc.tensor.matmul(st2[:, 0:512],
                                     lhsT=kT[0:64, 128 * i0:128 * (i0 + 1)],
                                     rhs=qT[0:64, 512 * j:512 * (j + 1)],
                                     tile_position=(0, 0))
                    nc.tensor.matmul(st2[:, 512:1024],
                                     lhsT=kT[64:128, 128 * i1:128 * (i1 + 1)],
                                     rhs=qT[64:128, 512 * j:512 * (j + 1)],
                                     tile_position=(64, 0))
                    pt = p_pool.tile([128, 1024], BF16, tag="p")
                    nc.scalar.activation(pt, st2, AF.Exp, bias=zbias, scale=SCALE)
                    pts[p] = pt
                if p == 3 and pending_epilogue is not None:
                    pending_epilogue()
                    pending_epilogue = None
                po = p - 1
                if po >= 0:
                    pt = pts.pop(po)
                    emit_o(2 * po, ot_lo, ot_hi, pt[:, 0:512])
                    emit_o(2 * po + 1, ot_lo, ot_hi, pt[:, 512:1024])
            # merge halves (+ l row); DVE may read only one PSUM input per op
            ot_sb = ep_pool.tile([65, 512], BF16, bufs=2, tag="ot_sb")
            nc.vector.tensor_copy(ot_sb, ot_lo)
            nc.vector.tensor_add(ot_sb, ot_sb, ot_hi)
            pending_epilogue = make_epilogue(j, ot_sb)
        pending_epilogue()


_NC_CACHE = {}


def _get_nc():
    if "nc" not in _NC_CACHE:
        _NC_CACHE["nc"] = build_kernel()
    return _NC_CACHE["nc"]


def build_in_maps(x, gamma, beta, wq, bq, wk, wv, bv, wo, bo):
    """Per-core NEFF input dicts plus (batch, rows) scatter info per core."""
    x = np.asarray(x, dtype=np.float32)
    shared = {
        "gamma": np.asarray(gamma, np.float32),
        "beta": np.asarray(beta, np.float32),
        "wq": np.asarray(wq, np.float32), "bq": np.asarray(bq, np.float32),
        "wk": np.asarray(wk, np.float32),
        "wv": np.asarray(wv, np.float32), "bv": np.asarray(bv, np.float32),
        "wo": np.asarray(wo, np.float32), "bo": np.asarray(bo, np.float32),
    }
    xf = x.reshape(B, S, C)
    in_maps = []
    scatter = []
    for core in range(8):
        b, h = core // 2, core % 2
        own = slice(h * SQ, (h + 1) * SQ)
        other = slice((1 - h) * SQ, (2 - h) * SQ)
        x_local = np.concatenate([xf[b][own], xf[b][other]], axis=0)
        in_maps.append({
            "xT": np.ascontiguousarray(x_local.T).astype(ml_dtypes.bfloat16),
            "x_q": np.ascontiguousarray(x_local[:SQ]),
            **shared,
        })
        scatter.append((b, np.arange(h * SQ, (h + 1) * SQ)))
    return in_maps, scatter


def _run(in_maps, scatter, **spmd_kwargs):
    nc = _get_nc()
    res = run_bass_kernel_spmd(nc, in_maps, core_ids=list(range(8)),
                               **spmd_kwargs)
    out = np.empty((B, S, C), np.float32)
    for core in range(8):
        b, rows = scatter[core]
        out[b][rows] = res.results[core]["out"]
    return out.reshape(B, H, W, C), res


def kernel(x, gamma, beta, wq, bq, wk, bk, wv, bv, wo, bo):
    # bk is provably a no-op: it shifts each query's scores by the constant
    # bk.q which softmax cancels, so it is not shipped to the device.
    in_maps, scatter = build_in_maps(x, gamma, beta, wq, bq, wk, wv, bv, wo, bo)
    out, _ = _run(in_maps, scatter)
    return out


# revision 19
# speedup vs baseline: 1.8739x; 1.0019x over previous
"""Trainium2 Bass kernel for nn_AttentionBlock (B=4, H=W=64, C=64, GROUPS=32).

Math (reference):
    hn = GroupNorm(x; gamma, beta, 32 groups, eps=1e-3)
    q = hn@wq+bq ; k = hn@wk+bk ; v = hn@wv+bv
    att = softmax(q k^T / 8) over the 4096 spatial positions
    out = x + (att @ v) @ wo + bo

Sharding: data-parallel, 2 cores per batch image, each core owns 2048 of the
4096 queries but holds the full key/value set for its batch. No collectives.

Per-core pipeline (fully fused on one NeuronCore):
  - xT [C=64, S=4096] arrives pre-transposed in bf16 (host does the cheap
    numpy transpose+cast), so channel-contraction matmuls need no on-chip
    transposes. x_q keeps the core's own query rows in fp32 for the residual.
  - GroupNorm stats via bn_stats/bn_aggr per channel on DVE, then tiny 0/1
    matmuls pair-combine channels into groups and expand back. The GN affine
    folds into the projection weights: W~ = diag(scale_c)@W, b~ = gnbias@W + b.
  - k-bias is dropped: it shifts each query's scores by a constant, which
    softmax cancels exactly.
  - Scores are computed transposed, ST[t, s] (keys on partitions), so exp(ST)
    feeds the att@v matmul directly as the moving operand - the attention
    matrix is never transposed. Score matmuls have K=64, so two key-chunks run
    CONCURRENTLY on the two halves of the PE array (row-tiling), with kT/qT
    mirrored onto partitions 64:127 by one SBUF-to-SBUF DMA each.
  - Softmax is max-free: |score| <= ~3 for unit-normal inputs so exp cannot
    overflow, and softmax(x) == softmax(x - max) exactly.
  - exp() runs one ACT instruction per chunk-pair over a 2-bank PSUM tile to
    amortize the ~352-cycle activation pipeline latency.
  - v gets an appended ones-column so att@v also accumulates the softmax
    denominator l[s]. att@v is split into two K=64 halves accumulating into
    two PSUM banks (summed by one DVE add at stripe end): the halves run on
    opposite array halves, letting LDWEIGHTS overlap in-flight matmuls.
  - The output projection runs on the unnormalized accumulator ((O/l)@wo ==
    (O@wo)/l), with an extra wo column passing l through; one reciprocal +
    fused multiply-add applies softmax normalization, residual and bo.
"""

import numpy as np
import ml_dtypes

import concourse.bass as bass
import concourse.tile as tile
from concourse import bacc, mybir
from concourse.bass_utils import run_bass_kernel_spmd

F32 = mybir.dt.float32
BF16 = mybir.dt.bfloat16
AF = mybir.ActivationFunctionType
ALU = mybir.AluOpType

B, H, W, C = 4, 64, 64, 64
S = H * W            # 4096 spatial positions per image
SQ = S // 2          # 2048 queries per core
EPS = 1e-3
N_CHUNK = S // 128   # 32 key chunks
NQ = SQ // 128       # 16 query chunks
N_STRIPE = SQ // 512  # 4 query stripes
SCALE = float(C) ** -0.5  # 0.125


def build_kernel():
    nc = bacc.Bacc("TRN2", target_bir_lowering=False, debug=False)

    xT_d = nc.dram_tensor("xT", [C, S], BF16, kind="ExternalInput")
    x_q = nc.dram_tensor("x_q", [SQ, C], F32, kind="ExternalInput")
    gamma = nc.dram_tensor("gamma", [C], F32, kind="ExternalInput")
    beta = nc.dram_tensor("beta", [C], F32, kind="ExternalInput")
    wq_d = nc.dram_tensor("wq", [C, C], F32, kind="ExternalInput")
    bq_d = nc.dram_tensor("bq", [C], F32, kind="ExternalInput")
    wk_d = nc.dram_tensor("wk", [C, C], F32, kind="ExternalInput")
    wv_d = nc.dram_tensor("wv", [C, C], F32, kind="ExternalInput")
    bv_d = nc.dram_tensor("bv", [C], F32, kind="ExternalInput")
    wo_d = nc.dram_tensor("wo", [C, C], F32, kind="ExternalInput")
    bo_d = nc.dram_tensor("bo", [C], F32, kind="ExternalInput")
    out_d = nc.dram_tensor("out", [SQ, C], F32, kind="ExternalOutput")

    with tile.TileContext(nc) as tc:
        _emit(nc, tc, xT_d.ap(), x_q.ap(), gamma.ap(), beta.ap(), wq_d.ap(),
              bq_d.ap(), wk_d.ap(), wv_d.ap(), bv_d.ap(), wo_d.ap(), bo_d.ap(),
              out_d.ap())
    nc.compile()
    return nc


def _emit(nc, tc, xT_d, x_q, gamma, beta, wq_d, bq_d, wk_d, wv_d, bv_d, wo_d,
          bo_d, out_d):
    from contextlib import ExitStack

    ctx = ExitStack()
    with ctx:
        const = ctx.enter_context(tc.tile_pool(name="const", bufs=1))
        big = ctx.enter_context(tc.tile_pool(name="big", bufs=1))
        tiny = ctx.enter_context(tc.tile_pool(name="tiny", bufs=1))

        # ---- big input DMAs first (sync/HWDGE ring), chunked so dependents
        # ---- can start early; partitions 64:127 mirror 0:63 for row-tiling ----
        xT = big.tile([128, S], BF16)
        for i in range(4):
            nc.sync.dma_start(out=xT[0:64, 1024 * i:1024 * (i + 1)],
                              in_=xT_d[:, 1024 * i:1024 * (i + 1)])
            nc.sync.dma_start(out=xT[64:128, 1024 * i:1024 * (i + 1)],
                              in_=xT[0:64, 1024 * i:1024 * (i + 1)])

        # ---- params via the scalar-engine HWDGE ring (parallel with sync) ----
        wq_aug = const.tile([65, 64], F32)   # [Wq ; bq]
        nc.scalar.dma_start(out=wq_aug[0:64, :], in_=wq_d)
        nc.scalar.dma_start(out=wq_aug[64:65, :], in_=bq_d.rearrange("(o c) -> o c", o=1))
        wk_sb = const.tile([128, 64], F32)
        nc.scalar.dma_start(out=wk_sb[0:64, :], in_=wk_d)
        nc.scalar.dma_start(out=wk_sb[64:128, :], in_=wk_d)
        wq_sb = const.tile([128, 64], F32)
        nc.scalar.dma_start(out=wq_sb[0:64, :], in_=wq_d)
        nc.scalar.dma_start(out=wq_sb[64:128, :], in_=wq_d)
        wv_sb = const.tile([128, 64], F32)
        nc.scalar.dma_start(out=wv_sb[0:64, :], in_=wv_d)
        nc.scalar.dma_start(out=wv_sb[64:128, :], in_=wv_d)
        wv_aug = const.tile([65, 65], F32)   # [Wv ; bv] plus e64 column
        nc.scalar.dma_start(out=wv_aug[0:64, 0:64], in_=wv_d)
        nc.scalar.dma_start(out=wv_aug[64:65, 0:64], in_=bv_d.rearrange("(o c) -> o c", o=1))
        nc.gpsimd.memset(wv_aug[0:64, 64:65], 0.0)
        nc.gpsimd.memset(wv_aug[64:65, 64:65], 1.0)
        # wo_aug = [wo ; bvo] plus e64 column that passes l through. Row 64
        # multiplies the l-row of the accumulator, so after the division by l
        # it contributes the constant row bvo = bv_total @ wo - this is how the
        # v-bias is applied without ever materializing it per-position.
        wo_aug = const.tile([65, 65], BF16)
        nc.gpsimd.dma_start(out=wo_aug[0:64, 0:64], in_=wo_d)  # SWDGE casts f32->bf16
        nc.gpsimd.memset(wo_aug[0:64, 64:65], 0.0)
        nc.gpsimd.memset(wo_aug[64:65, 64:65], 1.0)
        wo_sb = const.tile([64, 64], F32)
        nc.scalar.dma_start(out=wo_sb, in_=wo_d)
        gamma_col = const.tile([128, 1], F32)
        nc.scalar.dma_start(out=gamma_col[0:64, :], in_=gamma.rearrange("(c o) -> c o", o=1))
        nc.scalar.dma_start(out=gamma_col[64:128, :], in_=gamma.rearrange("(c o) -> c o", o=1))
        beta_col = const.tile([64, 1], F32)
        nc.scalar.dma_start(out=beta_col, in_=beta.rearrange("(c o) -> c o", o=1))
        bo_bcast = const.tile([128, 64], F32)
        nc.scalar.dma_start(out=bo_bcast, in_=bo_d.rearrange("(o c) -> o c", o=1).to_broadcast([128, 64]))

        zbias = const.tile([128, 1], F32)
        nc.gpsimd.memset(zbias, 0.0)
        # exp is the only ACT table set this kernel uses (rsqrt is done with a
        # Newton iteration on DVE); preload it while waiting on input DMAs.
        scratch1 = const.tile([1, 1], F32)
        nc.scalar.activation(scratch1, zbias[0:1, :], AF.Exp, bias=0.0, scale=1.0)

        # pair matrices: p64h[c,g] = 0.5 iff c//2 == g ; p32x64[g,c] = 1 iff c//2 == g
        p64h = const.tile([64, 32], F32)
        nc.gpsimd.memset(p64h, 0.5)
        nc.gpsimd.affine_select(out=p64h, in_=p64h, compare_op=ALU.is_ge,
                                fill=0.0, base=0, pattern=[[-2, 32]],
                                channel_multiplier=1)
        nc.gpsimd.affine_select(out=p64h, in_=p64h, compare_op=ALU.is_ge,
                                fill=0.0, base=1, pattern=[[2, 32]],
                                channel_multiplier=-1)
        p32x64 = const.tile([32, 64], F32)
        nc.gpsimd.memset(p32x64, 1.0)
        nc.gpsimd.affine_select(out=p32x64, in_=p32x64, compare_op=ALU.is_ge,
                                fill=0.0, base=0, pattern=[[1, 64]],
                                channel_multiplier=-2)
        nc.gpsimd.affine_select(out=p32x64, in_=p32x64, compare_op=ALU.is_ge,
                                fill=0.0, base=1, pattern=[[-1, 64]],
                                channel_multiplier=2)

        # ---- PSUM pools (8 banks: st 2x[128,1024] = 4, aux 4x one-bank) ----
        st_ps = ctx.enter_context(tc.tile_pool(name="st_ps", bufs=2, space="PSUM"))
        aux_ps = ctx.enter_context(tc.tile_pool(name="aux_ps", bufs=4, space="PSUM"))

        # ---- GroupNorm stats on DVE: per-channel mean/var over all 4096 ----
        bstats = tiny.tile([64, 8, 6], F32)
        for i in range(8):
            nc.vector.bn_stats(bstats[:, i, :], xT[0:64, 512 * i:512 * (i + 1)])
        mv = tiny.tile([64, 2], F32)
        nc.vector.bn_aggr(mv, bstats)
        packed64 = tiny.tile([64, 2], F32)        # [mean_c, E[x^2]_c]
        nc.vector.tensor_copy(packed64[:, 0:1], mv[:, 0:1])
        nc.vector.tensor_mul(packed64[:, 1:2], mv[:, 0:1], mv[:, 0:1])
        nc.vector.tensor_add(packed64[:, 1:2], packed64[:, 1:2], mv[:, 1:2])
        gpair = aux_ps.tile([32, 2], F32, tag="aux")  # group [mean, E[x^2]]
        nc.tensor.matmul(gpair, lhsT=p64h, rhs=packed64)
        gm = tiny.tile([32, 2], F32)
        nc.vector.tensor_copy(gm, gpair)
        var = tiny.tile([32, 1], F32)
        nc.vector.tensor_mul(var, gm[:, 0:1], gm[:, 0:1])
        nc.vector.tensor_sub(var, gm[:, 1:2], var)
        nc.vector.tensor_scalar_add(var, var, EPS)
        # rstd = rsqrt(var) entirely on DVE: quake-style bit seed + 3 Newton
        # steps (rel err < 1e-7 for any positive input) - keeps the scalar
        # engine's activation tables untouched for exp.
        U32 = mybir.dt.uint32
        magic = tiny.tile([32, 1], U32)
        nc.gpsimd.memset(magic, 0x5f3759df)
        ybits = tiny.tile([32, 1], U32)
        nc.vector.tensor_scalar(out=ybits, in0=var.bitcast(U32), scalar1=1,
                                scalar2=None, op0=ALU.logical_shift_right)
        nc.vector.tensor_sub(ybits, magic, ybits)
        rstd = ybits.bitcast(F32)
        t1 = tiny.tile([32, 1], F32)
        for _ in range(3):
            nc.vector.tensor_mul(t1, rstd, rstd)
            nc.vector.tensor_mul(t1, t1, var)
            nc.vector.tensor_scalar(out=t1, in0=t1, scalar1=-0.5, scalar2=1.5,
                                    op0=ALU.mult, op1=ALU.add)
            nc.vector.tensor_mul(rstd, rstd, t1)
        packed32 = tiny.tile([32, 2], F32)        # [rstd_g | mean_g]
        nc.vector.tensor_copy(packed32[:, 0:1], rstd)
        nc.vector.tensor_copy(packed32[:, 1:2], gm[:, 0:1])
        chan = aux_ps.tile([128, 2], F32, tag="aux")  # expand groups->channels,
        nc.tensor.matmul(chan[0:64, :], lhsT=p32x64, rhs=packed32)  # both halves
        nc.tensor.matmul(chan[64:128, :], lhsT=p32x64, rhs=packed32,
                         tile_position=(0, 64))
        scale_col = tiny.tile([128, 1], F32)      # rstd_g * gamma_c (mirrored)
        nc.vector.tensor_mul(scale_col, chan[:, 0:1], gamma_col)
        gnbias = tiny.tile([65, 1], F32)          # beta - mean*scale, aug 1
        nc.vector.tensor_mul(gnbias[0:64, :], chan[0:64, 1:2], scale_col[0:64, :])
        nc.vector.tensor_sub(gnbias[0:64, :], beta_col, gnbias[0:64, :])
        nc.gpsimd.memset(gnbias[64:65, :], 1.0)

        # ---- fold GN into projection weights (both halves in one op) ----
        wq_sc = tiny.tile([128, 64], BF16)
        nc.vector.tensor_scalar_mul(wq_sc, wq_sb, scale_col)
        wk_sc = tiny.tile([128, 64], BF16)
        nc.vector.tensor_scalar_mul(wk_sc, wk_sb, scale_col)
        wv_sc = tiny.tile([128, 64], BF16)
        nc.vector.tensor_scalar_mul(wv_sc, wv_sb, scale_col)

        bqp = aux_ps.tile([64, 1], F32, tag="aux")  # total q bias (column)
        nc.tensor.matmul(bqp, lhsT=wq_aug, rhs=gnbias)
        bq_col = tiny.tile([64, 1], F32)
        nc.vector.tensor_copy(bq_col, bqp)
        # bvo row for wo_aug: bvo = (gnbias@Wv + bv) @ wo, bounced through HBM
        # to land on partition 64 (engines are lane-locked; DMA is not). This
        # only gates the first output projection, well off the critical path.
        bvcp = aux_ps.tile([65, 1], F32, tag="aux")
        nc.tensor.matmul(bvcp, lhsT=wv_aug, rhs=gnbias)
        bv_col = tiny.tile([64, 1], F32)
        nc.vector.tensor_copy(bv_col, bvcp[0:64, :])
        bvop = aux_ps.tile([1, 64], F32, tag="aux")
        nc.tensor.matmul(bvop, lhsT=bv_col, rhs=wo_sb)
        bvo_row = tiny.tile([1, 64], F32)
        nc.vector.tensor_copy(bvo_row, bvop)
        bvo_stage = nc.dram_tensor("bvo_stage", [64], F32).ap()
        nc.sync.dma_start(out=bvo_stage.rearrange("(o c) -> o c", o=1), in_=bvo_row)
        nc.gpsimd.dma_start(out=wo_aug[64:65, 0:64],
                            in_=bvo_stage.rearrange("(o c) -> o c", o=1))

        # ---- projections: row-tiled pairs (even chunk on array rows 0:63,
        # ---- odd chunk on rows 64:127); kT/qT mirrored via per-chunk DMAs ----
        kT = big.tile([128, S], BF16)
        qT = big.tile([128, SQ], BF16)
        for j in range(4):
            kpa = aux_ps.tile([64, 512], F32, tag="aux")
            kpb = aux_ps.tile([64, 512], F32, tag="aux")
            ja, jb = 2 * j, 2 * j + 1
            nc.tensor.matmul(kpa, lhsT=wk_sc[0:64, :],
                             rhs=xT[0:64, 512 * ja:512 * (ja + 1)],
                             tile_position=(0, 0))
            nc.tensor.matmul(kpb, lhsT=wk_sc[64:128, :],
                             rhs=xT[64:128, 512 * jb:512 * (jb + 1)],
                             tile_position=(64, 0))
            nc.scalar.copy(out=kT[0:64, 512 * ja:512 * (ja + 1)], in_=kpa)
            nc.scalar.copy(out=kT[0:64, 512 * jb:512 * (jb + 1)], in_=kpb)
            nc.sync.dma_start(out=kT[64:128, 1024 * j:1024 * (j + 1)],
                              in_=kT[0:64, 1024 * j:1024 * (j + 1)])
            if j < 2:
                qpa = aux_ps.tile([64, 512], F32, tag="aux")
                qpb = aux_ps.tile([64, 512], F32, tag="aux")
                nc.tensor.matmul(qpa, lhsT=wq_sc[0:64, :],
                                 rhs=xT[0:64, 512 * ja:512 * (ja + 1)],
                                 tile_position=(0, 0))
                nc.tensor.matmul(qpb, lhsT=wq_sc[64:128, :],
                                 rhs=xT[64:128, 512 * jb:512 * (jb + 1)],
                                 tile_position=(64, 0))
                nc.vector.tensor_scalar_add(qT[0:64, 512 * ja:512 * (ja + 1)], qpa, bq_col)
                nc.vector.tensor_scalar_add(qT[0:64, 512 * jb:512 * (jb + 1)], qpb, bq_col)
                nc.sync.dma_start(out=qT[64:128, 1024 * j:1024 * (j + 1)],
                                  in_=qT[0:64, 1024 * j:1024 * (j + 1)])

        # v in natural [t, c] layout, one big tile; col 64 = ones (one memset)
        v_big = big.tile([128, N_CHUNK, 65], BF16)
        nc.gpsimd.memset(v_big[:, :, 64:65], 1.0)
        for p in range(N_CHUNK // 2):
            i0, i1 = 2 * p, 2 * p + 1
            vpa = aux_ps.tile([128, 64], F32, tag="aux")
            vpb = aux_ps.tile([128, 64], F32, tag="aux")
            nc.tensor.matmul(vpa, lhsT=xT[0:64, 128 * i0:128 * (i0 + 1)],
                             rhs=wv_sc[0:64, :], tile_position=(0, 0))
            nc.tensor.matmul(vpb, lhsT=xT[64:128, 128 * i1:128 * (i1 + 1)],
                             rhs=wv_sc[64:128, :], tile_position=(64, 0))
            nc.vector.tensor_copy(v_big[:, i0, 0:64], vpa)
            nc.vector.tensor_copy(v_big[:, i1, 0:64], vpb)

        # ---- residual base: x + bo (needed only by epilogues) ----
        xq_sb = big.tile([128, NQ, 64], F32)
        nc.sync.dma_start(out=xq_sb, in_=x_q.rearrange("(m p) c -> p m c", p=128))
        xb_sb = big.tile([128, NQ, 64], F32)
        for m in range(NQ):
            nc.vector.tensor_add(xb_sb[:, m, :], xq_sb[:, m, :], bo_bcast)

        # ---- main attention loop ----
        # Pairs of key chunks: the two K=64 score matmuls run concurrently on
        # the two row-halves of the PE array into the two banks of one PSUM
        # tile; exp covers both in one ACT instruction. att@v for each chunk
        # is split into two K=64 halves (lo/hi array rows) accumulating into
        # separate PSUM banks, summed once per stripe. All att@v work is
        # emitted one pair behind its exp so it never stalls the PE queue.
        p_pool = ctx.enter_context(tc.tile_pool(name="p_pool", bufs=4))
        ep_pool = ctx.enter_context(tc.tile_pool(name="ep_pool", bufs=3))
        N_PAIR = N_CHUNK // 2

        def emit_o(io, ot_lo, ot_hi, pt):
            nc.tensor.matmul(ot_lo, lhsT=v_big[0:64, io, :], rhs=pt[0:64, :],
                             tile_position=(0, 0),
                             start=(io == 0), stop=(io == N_CHUNK - 1))
            nc.tensor.matmul(ot_hi, lhsT=v_big[64:128, io, :], rhs=pt[64:128, :],
                             tile_position=(64, 0),
                             start=(io == 0), stop=(io == N_CHUNK - 1))

        def make_epilogue(j, ot_sb):
            def epi():
                res = ep_pool.tile([128, 4, 64], F32, tag="res", bufs=2)
                for m in range(4):
                    op = aux_ps.tile([128, 65], F32, tag="aux")
                    nc.tensor.matmul(op, lhsT=ot_sb[:, 128 * m:128 * (m + 1)],
                                     rhs=wo_aug)
                    rl = ep_pool.tile([128, 1], F32, tag="rl")
                    nc.vector.reciprocal(rl, op[:, 64:65])
                    nc.vector.scalar_tensor_tensor(out=res[:, m, :],
                                                   in0=op[:, 0:64],
                                                   scalar=rl,
                                                   in1=xb_sb[:, 4 * j + m, :],
                                                   op0=ALU.mult, op1=ALU.add)
                nc.sync.dma_start(
                    out=out_d[512 * j:512 * (j + 1), :].rearrange("(m p) c -> p m c", p=128),
                    in_=res)
            return epi

        pending_epilogue = None
        for j in range(N_STRIPE):
            ot_lo = aux_ps.tile([65, 512], F32, tag="aux")
            ot_hi = aux_ps.tile([65, 512], F32, tag="aux")
            pts = {}
            for p in range(N_PAIR + 1):
                if p < N_PAIR:
                    i0, i1 = 2 * p, 2 * p + 1
                    st2 = st_ps.tile([128, 1024], F32, tag="st")
                    nc.tensor.matmul(st2[:, 0:512],
                                     lhsT=kT[0:64, 128 * i0:128 * (i0 + 1)],
                                     rhs=qT[0:64, 512 * j:512 * (j + 1)],
                                     tile_position=(0, 0))
                    nc.tensor.matmul(st2[:, 512:1024],
                                     lhsT=kT[64:128, 128 * i1:128 * (i1 + 1)],
                                     rhs=qT[64:128, 512 * j:512 * (j + 1)],
                                     tile_position=(64, 0))
                    pt = p_pool.tile([128, 1024], BF16, tag="p")
                    nc.scalar.activation(pt, st2, AF.Exp, bias=zbias, scale=SCALE)
                    pts[p] = pt
                if p == 3 and pending_epilogue is not None:
                    pending_epilogue()
                    pending_epilogue = None
                po = p - 1
                if po >= 0:
                    pt = pts.pop(po)
                    emit_o(2 * po, ot_lo, ot_hi, pt[:, 0:512])
                    emit_o(2 * po + 1, ot_lo, ot_hi, pt[:, 512:1024])
            # merge halves (+ l row); DVE may read only one PSUM input per op
            ot_sb = ep_pool.tile([65, 512], BF16, bufs=2, tag="ot_sb")
            nc.vector.tensor_copy(ot_sb, ot_lo)
            nc.vector.tensor_add(ot_sb, ot_sb, ot_hi)
            pending_epilogue = make_epilogue(j, ot_sb)
        pending_epilogue()


_NC_CACHE = {}


def _get_nc():
    if "nc" not in _NC_CACHE:
        _NC_CACHE["nc"] = build_kernel()
    return _NC_CACHE["nc"]


def build_in_maps(x, gamma, beta, wq, bq, wk, wv, bv, wo, bo):
    """Per-core NEFF input dicts plus (batch, rows) scatter info per core."""
    x = np.asarray(x, dtype=np.float32)
    shared = {
        "gamma": np.asarray(gamma, np.float32),
        "beta": np.asarray(beta, np.float32),
        "wq": np.asarray(wq, np.float32), "bq": np.asarray(bq, np.float32),
        "wk": np.asarray(wk, np.float32),
        "wv": np.asarray(wv, np.float32), "bv": np.asarray(bv, np.float32),
        "wo": np.asarray(wo, np.float32), "bo": np.asarray(bo, np.float32),
    }
    xf = x.reshape(B, S, C)
    in_maps = []
    scatter = []
    for core in range(8):
        b, h = core // 2, core % 2
        own = slice(h * SQ, (h + 1) * SQ)
        other = slice((1 - h) * SQ, (2 - h) * SQ)
        x_local = np.concatenate([xf[b][own], xf[b][other]], axis=0)
        in_maps.append({
            "xT": np.ascontiguousarray(x_local.T).astype(ml_dtypes.bfloat16),
            "x_q": np.ascontiguousarray(x_local[:SQ]),
            **shared,
        })
        scatter.append((b, np.arange(h * SQ, (h + 1) * SQ)))
    return in_maps, scatter


def _run(in_maps, scatter, **spmd_kwargs):
    nc = _get_nc()
    res = run_bass_kernel_spmd(nc, in_maps, core_ids=list(range(8)),
                               **spmd_kwargs)
    out = np.empty((B, S, C), np.float32)
    for core in range(8):
        b, rows = scatter[core]
        out[b][rows] = res.results[core]["out"]
    return out.reshape(B, H, W, C), res


def kernel(x, gamma, beta, wq, bq, wk, bk, wv, bv, wo, bo):
    # bk is provably a no-op: it shifts each query's scores by the constant
    # bk.q which softmax cancels, so it is not shipped to the device.
    in_maps, scatter = build_in_maps(x, gamma, beta, wq, bq, wk, wv, bv, wo, bo)
    out, _ = _run(in_maps, scatter)
    return out
